# revision 2
# baseline (speedup 1.0000x reference)
"""BrainGCN on 8 Trainium2 NeuronCores (Bass/Tile, SPMD).

kernel(**inputs) takes FULL unsharded inputs, returns the full (G,) output.

Sharding: N nodes in 8 contiguous shards (SH=N/8); edges live on the core
owning their dst node, grouped into 128-node dst windows.  Tiles of 128
edges are classified by (window j, src%NPAR parity r, src-chunk q) where q
indexes NQ slices of every shard (for AllGather pipelining); per-(j,r) tile
counts are equalized across cores so one SPMD program serves all 8.

Unified one-hot: both layers use per-edge coefficient w'_e = w_e *
rsqrt(deg[src]).  The t2 table is h1@W2p WITHOUT the dst dinv fold, so a
single one-hot (iota==dloc)*w' drives both the L1 aggregation (rhs=onehot,
lhsT=host-pregathered x[src] rows) and the L2 aggregation (lhsT=onehot,
rhs=per-edge gathered t2 rows).  One-hots are built twice (P1 and L2) on a
DVE/Pool/ACT mix; self-loops enter via diag(dinv) matmuls.

w' comes from host-pregathered per-edge degree-slot rows (wde) reduced on
device (bf16) + Rsqrt; all FLOPs stay on device (host only permutes/pads
input data).

L2 messages: t2 stored fp8e4 [N,64] (64B rows); SWDGE bulk dma_gather with
256B elems (4 rows), NPAR=4 src-parity classes picking the row via shifted
table views; idx16 = src//4.  The AllGather runs in NQ=4 window-aligned
chunks, each fired as soon as every core has written that slice of its t2
shard, so gathers for chunk q overlap P1 compute of later chunks.

Scatter-add = one-hot matmuls into 49 concurrently-open PSUM accumulation
chains (one per dst window, skip_group_check).  Pooling = one-hot(batch)
matmul with a ones column; partial [G, H+1] pools are AllGathered (cheaper
than AllReduce) and summed on-device; the tiny MLP head is replicated.
"""
import math
from contextlib import ExitStack

import numpy as np
import ml_dtypes

import concourse.bass as bass
import concourse.bacc as bacc
import concourse.tile as tile
import concourse.mybir as mybir
from concourse import library_config
from concourse.bass_utils import run_bass_kernel_spmd

F32 = mybir.dt.float32
BF16 = mybir.dt.bfloat16
FP8 = mybir.dt.float8e4
I16 = mybir.dt.int16
AL = mybir.AluOpType
ACTF = mybir.ActivationFunctionType
BF = ml_dtypes.bfloat16
F8 = ml_dtypes.float8_e4m3

N_CORES = 8
WIN = 128
EPS = 1e-5
NQ = 4            # AllGather chunks (window-aligned slices of each shard)
NPAR = 4          # src parity classes (fp8 rows: 256B elem = 4 rows)
CALL_TILES = 32   # tiles per dma_gather call (legacy, unused)
MSGS_TILES = 64   # tiles buffered per (q, group) msgs buffer
JBLOCK = 5        # windows per wde streaming chunk


def _prep_inputs(inputs: dict):
    x = np.asarray(inputs["x"], np.float32)
    ei = np.asarray(inputs["edge_index"])
    ew = np.asarray(inputs["edge_weight"], np.float32)
    batch = np.asarray(inputs["batch"]).astype(np.int64)
    N, IN_DIM = x.shape
    HID = np.asarray(inputs["W1"]).shape[1]
    assert N % N_CORES == 0
    SH = N // N_CORES
    NWIN = math.ceil(SH / WIN)

    # window-aligned q boundaries (in windows, then node rows of each shard).
    # Front-loaded: a tile of 128 edges straddles into the next chunk, so
    # early chunks get extra windows to keep per-q gather volumes balanced.
    wq = [0, 15, 28, 41, NWIN] if NWIN == 49 else \
        [round(q * NWIN / NQ) for q in range(NQ + 1)]
    rq = [min(w * WIN, SH) for w in wq]

    # degree rows incl self-loop weight 1 (self-loops excluded from edges)
    dstA = np.concatenate([np.asarray(ei[1]), np.arange(N)]).astype(np.int64)
    wA = np.concatenate([ew, np.ones(N, np.float32)]).astype(np.float32)
    orderA = np.argsort(dstA, kind="stable")
    dsA, wsA = dstA[orderA], wA[orderA]
    countsA = np.bincount(dstA, minlength=N)
    DSLOT = int(countsA.max())
    rowptrA = np.zeros(N + 1, np.int64)
    np.cumsum(countsA, out=rowptrA[1:])
    wdeg_full = np.zeros((N, DSLOT), np.float32)
    slotA = np.arange(len(dsA)) - rowptrA[dsA]
    wdeg_full[dsA, slotA] = wsA

    src = np.asarray(ei[0]).astype(np.int64)
    dst = np.asarray(ei[1]).astype(np.int64)
    w = ew.astype(np.float32)
    order = np.argsort(dst, kind="stable")
    ds, ss, ws = dst[order], src[order], w[order]
    qs = np.searchsorted(rq, ss % SH, side="right") - 1  # src q-chunk
    # chunk-major table layout [q][core][local row][h]: each chunk's
    # AllGather output is one contiguous block (BIR requires contiguity).
    Bq = [rq[q + 1] - rq[q] for q in range(NQ)]
    qbase = np.concatenate([[0], np.cumsum([N_CORES * b for b in Bq])])
    c_src = ss // SH
    i_src = ss % SH
    rows = (qbase[qs] + c_src * np.array(Bq)[qs]
            + (i_src - np.array(rq)[qs])).astype(np.int64)
    rs = rows % NPAR
    counts = np.bincount(dst, minlength=N)
    rowptr = np.zeros(N + 1, np.int64)
    np.cumsum(counts, out=rowptr[1:])

    # per-core (window, parity) segments, sorted by q inside
    seg = {}
    cnt = np.zeros((N_CORES, NWIN, NPAR), np.int64)
    for c in range(N_CORES):
        for j in range(NWIN):
            lo = c * SH + j * WIN
            hi = min(c * SH + (j + 1) * WIN, (c + 1) * SH)
            a, b = int(rowptr[lo]), int(rowptr[hi])
            sl = slice(a, b)
            for r in range(NPAR):
                idxr = np.nonzero(rs[sl] == r)[0]
                idxr = idxr[np.argsort(qs[sl][idxr], kind="stable")]
                seg[(c, j, r)] = (lo, idxr)
                cnt[c, j, r] = len(idxr)

    ntile = np.zeros((NWIN, NPAR), np.int64)
    for j in range(NWIN):
        for r in range(NPAR):
            ntile[j, r] = -(-int(cnt[:, j, r].max()) // 128)
    T = int(ntile.sum())

    # tile q-class = max over cores of (q of last edge in tile)
    tclass = {}
    for j in range(NWIN):
        for r in range(NPAR):
            for t in range(int(ntile[j, r])):
                qmax = 0
                for c in range(N_CORES):
                    lo, idxr = seg[(c, j, r)]
                    if t * 128 >= len(idxr):
                        continue
                    e1 = min((t + 1) * 128, len(idxr)) - 1
                    a = int(rowptr[lo])
                    qmax = max(qmax, int(qs[a + idxr[e1]]))
                tclass[(j, r, t)] = qmax

    # window-major column order (for edloc/ewt/xg/wde/wprime)
    colW = {}
    c_acc = 0
    for j in range(NWIN):
        for r in range(NPAR):
            for t in range(int(ntile[j, r])):
                colW[(j, r, t)] = c_acc
                c_acc += 1
    assert c_acc == T

    # gather order: (q, window-group, r, j, t).  One dma_gather call per
    # (q, group, r); within a (q, group) the matmul sweep is window-major so
    # each PSUM bank has at most one open accumulation chain (hardware
    # corrupts interleaved chains within a bank).  Groups are greedy runs of
    # windows holding at most MSGS_TILES q-class tiles.
    qtiles_per_win = np.zeros((NQ, NWIN), np.int64)
    for (j, r, t), q in tclass.items():
        qtiles_per_win[q, j] += 1
    groups = []            # (q, jlo, jhi)
    for q in range(NQ):
        jlo = 0
        while jlo < NWIN:
            jhi, tot = jlo, 0
            while jhi < NWIN and tot + qtiles_per_win[q, jhi] <= MSGS_TILES:
                tot += qtiles_per_win[q, jhi]
                jhi += 1
            assert jhi > jlo
            groups.append((q, jlo, jhi))
            jlo = jhi
    gorder = []
    gcalls = []            # (gi, r, k_lo, k_hi)
    qb_tiles = {}          # (gi, j) -> [(k, colW)]
    for gi, (q, jlo, jhi) in enumerate(groups):
        for r in range(NPAR):
            k_lo = len(gorder)
            for j in range(jlo, jhi):
                for t in range(int(ntile[j, r])):
                    if tclass[(j, r, t)] == q:
                        gorder.append((j, r, t))
                        qb_tiles.setdefault((gi, j), []).append(
                            (len(gorder) - 1, colW[(j, r, t)]))
            if len(gorder) > k_lo:
                gcalls.append((gi, r, k_lo, len(gorder)))
    assert len(gorder) == T
    kmap = {jrt: k for k, jrt in enumerate(gorder)}

    win_tiles = [[] for _ in range(NWIN)]
    for j in range(NWIN):
        for r in range(NPAR):
            for t in range(int(ntile[j, r])):
                win_tiles[j].append((colW[(j, r, t)], kmap[(j, r, t)]))
    tile_info = [(j, colW[(j, r, t)]) for (j, r, t) in gorder]

    # --- per-core arrays ---
    edloc = np.full((N_CORES, 128, T), 999.0, np.float32)
    ewt = np.zeros((N_CORES, 128, T), np.float32)
    xg = np.zeros((N_CORES, 128, T * IN_DIM), BF)
    wde = np.zeros((N_CORES, 128, T * DSLOT), BF)
    idx16 = np.zeros((N_CORES, 16, T * 8), np.int16)

    for c in range(N_CORES):
        for j in range(NWIN):
            for r in range(NPAR):
                lo, idxr = seg[(c, j, r)]
                a = int(rowptr[lo])
                n = len(idxr)
                nt = int(ntile[j, r])
                cap = nt * 128
                s_pad = np.zeros(cap, np.int64)
                row_pad = np.zeros(cap, np.int64)
                d_pad = np.full(cap, 999.0, np.float32)
                w_pad = np.zeros(cap, np.float32)
                s_pad[:n] = ss[a + idxr]
                row_pad[:n] = rows[a + idxr]
                d_pad[:n] = (ds[a + idxr] - lo).astype(np.float32)
                w_pad[:n] = ws[a + idxr]
                for t in range(nt):
                    cw = colW[(j, r, t)]
                    k = kmap[(j, r, t)]
                    blk = slice(t * 128, (t + 1) * 128)
                    edloc[c, :, cw] = d_pad[blk]
                    ewt[c, :, cw] = w_pad[blk]
                    xs = x[s_pad[blk]].astype(np.float32)
                    if t * 128 >= n:
                        xs[:] = 0.0
                    elif (t + 1) * 128 > n:
                        xs[n - t * 128:] = 0.0
                    xg[c, :, cw * IN_DIM:(cw + 1) * IN_DIM] = xs.astype(BF)
                    wd = wdeg_full[s_pad[blk]].copy()
                    if t * 128 >= n:
                        wd[:] = 0.0
                        wd[:, 0] = 1.0
                    elif (t + 1) * 128 > n:
                        wd[n - t * 128:] = 0.0
                        wd[n - t * 128:, 0] = 1.0
                    wde[c, :, cw * DSLOT:(cw + 1) * DSLOT] = wd.astype(BF)
                    iv = (row_pad[blk] // NPAR).astype(np.int16)
                    if t * 128 >= n:
                        iv[:] = 0
                    elif (t + 1) * 128 > n:
                        iv[n - t * 128:] = 0
                    iv2 = iv.reshape(8, 16)  # [p//16, p%16]
                    idx16[c, :, k * 8:(k + 1) * 8] = iv2.T

    PADN = NWIN * WIN

    def win_major(a2d, dt=np.float32):
        S = a2d.shape[1]
        assert a2d.shape[0] == PADN
        return np.ascontiguousarray(
            a2d.reshape(NWIN, WIN, S).transpose(1, 0, 2)
            .reshape(WIN, NWIN * S)).astype(dt)

    iota = np.tile(np.arange(128, dtype=np.float32), (128, 1))
    ident = np.eye(128, dtype=np.float32)

    in_maps = []
    for c in range(N_CORES):
        wc = np.zeros((PADN, DSLOT), np.float32)
        wc[:SH] = wdeg_full[c * SH:(c + 1) * SH]
        wc[SH:, 0] = 1.0
        bv = np.full((PADN, 1), 999.0, np.float32)
        bv[:SH, 0] = batch[c * SH:(c + 1) * SH].astype(np.float32)
        xo = np.zeros((PADN, IN_DIM), np.float32)
        xo[:SH] = x[c * SH:(c + 1) * SH]
        in_maps.append({
            "edloc": edloc[c], "ewt": ewt[c].astype(BF),
            "xg": xg[c], "wde": wde[c],
            "idx16": np.tile(idx16[c], (8, 1)),
            "xnm": win_major(xo, BF),
            "wdeg": win_major(wc, BF), "batchv": win_major(bv),
            "iota": iota.astype(BF), "ident": ident,
            "W1": np.asarray(inputs["W1"], np.float32),
            "W2": np.asarray(inputs["W2"], np.float32),
            "g1": np.asarray(inputs["bn1_gamma"], np.float32).reshape(1, HID),
            "be1": np.asarray(inputs["bn1_beta"], np.float32).reshape(1, HID),
            "m1": np.asarray(inputs["bn1_mean"], np.float32).reshape(1, HID),
            "v1": np.asarray(inputs["bn1_var"], np.float32).reshape(1, HID),
            "b1": np.asarray(inputs["b1"], np.float32).reshape(1, HID),
            "g2": np.asarray(inputs["bn2_gamma"], np.float32).reshape(1, HID),
            "be2": np.asarray(inputs["bn2_beta"], np.float32).reshape(1, HID),
            "m2": np.asarray(inputs["bn2_mean"], np.float32).reshape(1, HID),
            "v2": np.asarray(inputs["bn2_var"], np.float32).reshape(1, HID),
            "b2": np.asarray(inputs["b2"], np.float32).reshape(1, HID),
            "lin1W": np.asarray(inputs["lin1_W"], np.float32),
            "lin1b": np.asarray(inputs["lin1_b"], np.float32).reshape(-1, 1),
            "lin2W": np.asarray(inputs["lin2_W"], np.float32),
            "lin2b": np.asarray(inputs["lin2_b"], np.float32).reshape(1, 1),
        })

    meta = dict(N=N, G=128, IN_DIM=IN_DIM, HID=HID, SH=SH, NWIN=NWIN,
                DSLOT=DSLOT, T=T, wq=wq, rq=rq,
                qbase=[int(x) for x in qbase], Bq=Bq,
                win_tiles=win_tiles, tile_info=tile_info,
                gcalls=gcalls, qb_tiles=qb_tiles, groups=groups)
    return in_maps, meta


def _build_nc(meta, no_collectives=False, no_gather=False):
    N, IN_DIM, HID = meta["N"], meta["IN_DIM"], meta["HID"]
    SH, NWIN, DSLOT, T = meta["SH"], meta["NWIN"], meta["DSLOT"], meta["T"]
    wq, rq, qbase = meta["wq"], meta["rq"], meta["qbase"]
    win_tiles = meta["win_tiles"]
    tile_info = meta["tile_info"]
    gcalls, qb_tiles, groups = meta["gcalls"], meta["qb_tiles"], meta["groups"]
    H2 = HID // 2

    nc = bacc.Bacc("TRN2", target_bir_lowering=False, debug=False,
                   num_devices=N_CORES)
    d_edloc = nc.dram_tensor("edloc", [128, T], F32, kind="ExternalInput")
    d_ewt = nc.dram_tensor("ewt", [128, T], BF16, kind="ExternalInput")
    d_xg = nc.dram_tensor("xg", [128, T * IN_DIM], BF16, kind="ExternalInput")
    d_wde = nc.dram_tensor("wde", [128, T * DSLOT], BF16, kind="ExternalInput")
    d_idx = nc.dram_tensor("idx16", [128, T * 8], I16, kind="ExternalInput")
    d_xnm = nc.dram_tensor("xnm", [128, NWIN * IN_DIM], BF16,
                           kind="ExternalInput")
    d_wdeg = nc.dram_tensor("wdeg", [128, NWIN * DSLOT], BF16,
                            kind="ExternalInput")
    d_batch = nc.dram_tensor("batchv", [128, NWIN], F32, kind="ExternalInput")
    d_iota = nc.dram_tensor("iota", [128, 128], BF16, kind="ExternalInput")
    d_ident = nc.dram_tensor("ident", [128, 128], F32, kind="ExternalInput")
    d_W1 = nc.dram_tensor("W1", [IN_DIM, HID], F32, kind="ExternalInput")
    d_W2 = nc.dram_tensor("W2", [HID, HID], F32, kind="ExternalInput")
    bn_names = ["g1", "be1", "m1", "v1", "b1", "g2", "be2", "m2", "v2", "b2"]
    d_bn = {k: nc.dram_tensor(k, [1, HID], F32, kind="ExternalInput")
            for k in bn_names}
    d_lin1W = nc.dram_tensor("lin1W", [HID, H2], F32, kind="ExternalInput")
    d_lin1b = nc.dram_tensor("lin1b", [H2, 1], F32, kind="ExternalInput")
    d_lin2W = nc.dram_tensor("lin2W", [H2, 1], F32, kind="ExternalInput")
    d_lin2b = nc.dram_tensor("lin2b", [1, 1], F32, kind="ExternalInput")
    d_out = nc.dram_tensor("out", [1, 128], F32, kind="ExternalOutput")

    rg = [list(range(N_CORES))]

    with tile.TileContext(nc) as tc, ExitStack() as ctx:
        constp = ctx.enter_context(tc.tile_pool(name="const", bufs=1))
        metap = ctx.enter_context(tc.tile_pool(name="meta", bufs=1))
        wdep = ctx.enter_context(tc.tile_pool(name="wdep", bufs=2))
        msgsp = ctx.enter_context(tc.tile_pool(name="msgs", bufs=2))
        wdep1 = ctx.enter_context(tc.tile_pool(name="wdep1", bufs=1))
        ohp = ctx.enter_context(tc.tile_pool(name="oh", bufs=16))
        epp = ctx.enter_context(tc.tile_pool(name="ep", bufs=4))
        vecp = ctx.enter_context(tc.tile_pool(name="vec", bufs=1))
        psA = ctx.enter_context(tc.tile_pool(name="psA", bufs=1,
                                              space="PSUM"))
        dram = ctx.enter_context(tc.tile_pool(name="dram", bufs=1,
                                              space="DRAM"))

        nc.gpsimd.load_library(library_config.mlp)

        # ---- constants and small inputs ----
        iota = constp.tile([128, 128], BF16)
        nc.sync.dma_start(iota[:], d_iota.ap())
        ident = constp.tile([128, 128], F32)
        nc.sync.dma_start(ident[:], d_ident.ap())
        ones1 = constp.tile([1, 128], F32)
        nc.vector.memset(ones1[:], 1.0)

        sb_edloc = metap.tile([128, T], F32)
        nc.sync.dma_start(sb_edloc[:], d_edloc.ap())
        sb_ewt = metap.tile([128, T], BF16)
        nc.sync.dma_start(sb_ewt[:], d_ewt.ap())
        sb_xg = metap.tile([128, T * IN_DIM], BF16)
        nc.sync.dma_start(sb_xg[:], d_xg.ap())
        sb_idx = metap.tile([128, T * 8], I16)
        nc.sync.dma_start(sb_idx[:], d_idx.ap())
        sb_xnm = metap.tile([128, NWIN * IN_DIM], BF16)
        nc.sync.dma_start(sb_xnm[:], d_xnm.ap())
        sb_wdeg = metap.tile([128, NWIN * DSLOT], BF16)
        nc.sync.dma_start(sb_wdeg[:], d_wdeg.ap())
        sb_batch = metap.tile([128, NWIN], F32)
        nc.sync.dma_start(sb_batch[:], d_batch.ap())
        sb_W1 = constp.tile([IN_DIM, HID], F32)
        nc.sync.dma_start(sb_W1[:], d_W1.ap())
        sb_W2 = constp.tile([HID, HID], F32)
        nc.sync.dma_start(sb_W2[:], d_W2.ap())
        sb_bn = {}
        for k in bn_names:
            sb_bn[k] = vecp.tile([1, HID], F32, tag=k, name="sb_" + k)
            nc.sync.dma_start(sb_bn[k][:], d_bn[k].ap())
        sb_lin1W = constp.tile([HID, H2], F32)
        sb_lin1b = constp.tile([H2, 1], F32)
        sb_lin2W = constp.tile([H2, 1], F32)
        sb_lin2b = constp.tile([1, 1], F32)
        nc.sync.dma_start(sb_lin1W[:], d_lin1W.ap())
        nc.sync.dma_start(sb_lin1b[:], d_lin1b.ap())
        nc.sync.dma_start(sb_lin2W[:], d_lin2W.ap())
        nc.sync.dma_start(sb_lin2b[:], d_lin2b.ap())

        # ---- BN folds ----
        def bn_fold(g, be, m, v, b):
            bns = vecp.tile([1, HID], F32, tag="bns" + g, name="bns" + g)
            nc.vector.tensor_scalar(out=bns[:], in0=sb_bn[v][:], scalar1=EPS,
                                    scalar2=None, op0=AL.add)
            nc.scalar.activation(bns[:], bns[:], ACTF.Sqrt)
            nc.vector.reciprocal(bns[:], bns[:])
            nc.vector.tensor_tensor(out=bns[:], in0=bns[:], in1=sb_bn[g][:],
                                    op=AL.mult)
            cc = vecp.tile([1, HID], F32, tag="c" + g, name="c" + g)
            nc.vector.tensor_tensor(out=cc[:], in0=sb_bn[b][:],
                                    in1=sb_bn[m][:], op=AL.subtract)
            nc.vector.tensor_tensor(out=cc[:], in0=cc[:], in1=bns[:],
                                    op=AL.mult)
            nc.vector.tensor_tensor(out=cc[:], in0=cc[:], in1=sb_bn[be][:],
                                    op=AL.add)
            return bns, cc

        bns1, c1v = bn_fold("g1", "be1", "m1", "v1", "b1")
        bns2, c2v = bn_fold("g2", "be2", "m2", "v2", "b2")

        # PSUM banks: two rotating L2 chain banks (one open accumulation
        # chain per bank at a time — interleaved chains within a bank are
        # corrupted by hardware), a pooling bank, and two P1 transient banks.
        chainb = [psA.tile([128, HID], F32, tag=f"ch{i}", name=f"ch{i}")[:]
                  for i in range(2)]
        poolbk = psA.tile([128, HID + 1], F32, tag="poolb", name="poolb")
        pool_ps = poolbk[:]
        psM1 = psA.tile([128, 512], F32, tag="psM1", name="psM1")
        psM2 = psA.tile([128, 512], F32, tag="psM2", name="psM2")

        def bcast128(vec, tag):
            ps = psM2[:, 128:128 + HID]
            nc.tensor.matmul(out=ps, lhsT=ones1[:], rhs=vec[:],
                             start=True, stop=True)
            sb = constp.tile([128, HID], F32, tag=tag, name="sb" + tag)
            nc.vector.tensor_copy(sb[:], ps)
            return sb

        c1_b = bcast128(c1v, "c1b")
        c2_b = bcast128(c2v, "c2b")

        def wfold(sb_W, bns, parts, tag):
            one_r = constp.tile([1, parts], F32, tag="oner" + tag,
                                name="oner" + tag)
            nc.vector.memset(one_r[:], 1.0)
            ps = psM2[0:parts, 128:128 + HID]
            nc.tensor.matmul(out=ps, lhsT=one_r[:], rhs=bns[:],
                             start=True, stop=True)
            wp = constp.tile([parts, HID], F32, tag="wp" + tag,
                             name="wp" + tag)
            nc.vector.tensor_tensor(out=wp[:], in0=sb_W[:], in1=ps[:],
                                    op=AL.mult)
            return wp

        W1p = wfold(sb_W1, bns1, IN_DIM, "1")
        W2p = wfold(sb_W2, bns2, HID, "2")

        # ---- dst dinv + persisted diag / pooling one-hots ----
        dinv = constp.tile([128, NWIN], F32)
        with nc.allow_low_precision(reason="deg sums fit bf16"):
            degd = epp.tile([128, NWIN], BF16, tag="degd", name="degd")
            nc.vector.tensor_reduce(
                out=degd[:].rearrange("p (j s) -> p j s", s=1),
                in_=sb_wdeg[:].rearrange("p (j s) -> p j s", s=DSLOT),
                op=AL.add, axis=mybir.AxisListType.X)
        nc.scalar.activation(dinv[:], degd[:], ACTF.Sqrt)
        nc.vector.reciprocal(dinv[:], dinv[:])

        dgall = constp.tile([128, NWIN * 128], BF16)

        # ---- w' = ewt * rsqrt(deg[src]) (streamed with P1 below) ----
        wprime = metap.tile([128, T], F32)
        negdloc = metap.tile([128, T], F32)
        negwp = metap.tile([128, T], F32)
        nc.vector.tensor_scalar(out=negdloc[:], in0=sb_edloc[:],
                                scalar1=-1.0, scalar2=None, op0=AL.mult)

        # t2 table (fp8) in DRAM
        t2_sh = dram.tile([SH, HID], FP8)
        t2_full = dram.tile([N + NPAR, HID], FP8)
        t2flat = t2_full[:].rearrange("n h -> (n h)")
        NROW4 = N // NPAR
        t2vr = [t2flat[r * HID: r * HID + NROW4 * NPAR * HID]
                .rearrange("(m k) -> m k", k=NPAR * HID)
                for r in range(NPAR)]
        zrow = constp.tile([NPAR, HID], FP8)
        nc.vector.memset(zrow[:], 0.0)
        nc.sync.dma_start(t2_full[N:N + NPAR, :], zrow[:])

        t2keep = metap.tile([128, NWIN * HID], BF16)
        h2acc = metap.tile([128, NWIN * HID], F32)

        # one-hot build engine schedule.  Pool builds only in early P1
        # windows (its in-order queue must reach the L2 gathers before AG0
        # completes); ACT carries a slice everywhere; DVE the rest.
        def build_oh(dst_ap, col, k, eng):
            if eng == "A":
                tt = ohp.tile([128, 128], BF16, tag="att", name="att")
                nc.scalar.activation(tt[:], iota[:], ACTF.Abs,
                                     bias=negdloc[:, col:col + 1])
                nc.scalar.activation(dst_ap, tt[:], ACTF.Relu,
                                     bias=wprime[:, col:col + 1],
                                     scale=negwp[:, col:col + 1])
            elif eng == "P":
                nc.gpsimd.tensor_scalar(
                    out=dst_ap, in0=iota[:],
                    scalar1=sb_edloc[:, col:col + 1],
                    scalar2=wprime[:, col:col + 1],
                    op0=AL.is_equal, op1=AL.mult)
            else:
                nc.vector.tensor_scalar(
                    out=dst_ap, in0=iota[:],
                    scalar1=sb_edloc[:, col:col + 1],
                    scalar2=wprime[:, col:col + 1],
                    op0=AL.is_equal, op1=AL.mult)

        def p1_eng(j, k):
            return "A" if k % 5 == 1 else "D"

        def l2_eng(k):
            return "A" if k % 5 == 1 else "D"

        # ---- P1: per-block wde streaming, per-window L1 + t2, chunked AG ----
        # window-major columns: window j covers [wstart[j], wstart[j+1])
        wstart = [0] * (NWIN + 1)
        for j in range(NWIN):
            wstart[j + 1] = wstart[j] + len(win_tiles[j])
        maxbt = max(wstart[min(j + JBLOCK, NWIN)] - wstart[j]
                    for j in range(NWIN))

        q_of_block_end = {}
        for q in range(NQ):
            q_of_block_end[wq[q + 1] - 1] = q

        def p1_front(j):
            # diag one-hot JIT, then the L1 accumulation chain
            nc.vector.tensor_scalar(
                out=dgall[:, j * 128:(j + 1) * 128], in0=ident[:],
                scalar1=dinv[:, j:j + 1], scalar2=None, op0=AL.mult)
            psM = psM1 if j % 2 == 0 else psM2
            acc5 = psM[0:IN_DIM, 0:128]
            first = True
            for (col, k) in win_tiles[j]:
                oh = ohp.tile([128, 128], BF16, tag="oh", name="oh")
                build_oh(oh[:], col, k, p1_eng(j, k))
                nc.tensor.matmul(
                    out=acc5,
                    lhsT=sb_xg[:, col * IN_DIM:(col + 1) * IN_DIM],
                    rhs=oh[:], start=first, stop=False)
                first = False
            nc.tensor.matmul(
                out=acc5,
                lhsT=sb_xnm[:, j * IN_DIM:(j + 1) * IN_DIM],
                rhs=dgall[:, j * 128:(j + 1) * 128],
                start=first, stop=True)

        def p1_tail(j):
            wlen = min(WIN, SH - j * WIN)
            psM = psM1 if j % 2 == 0 else psM2
            acc5 = psM[0:IN_DIM, 0:128]
            agg5 = epp.tile([IN_DIM, 128], F32, tag="agg5", name="agg5")
            nc.scalar.activation(agg5[:], acc5, ACTF.Copy)
            ps1 = psM[:, 128:128 + HID]
            nc.tensor.matmul(out=ps1, lhsT=agg5[:], rhs=W1p[:],
                             start=True, stop=True)
            h1 = epp.tile([128, HID], F32, tag="h1", name="h1")
            nc.vector.scalar_tensor_tensor(
                out=h1[:], in0=ps1, scalar=dinv[:, j:j + 1],
                in1=c1_b[:], op0=AL.mult, op1=AL.add)
            pT = psM[0:HID, 192:320]
            nc.tensor.transpose(out=pT, in_=h1[:], identity=ident[:])
            # relu(x)^T == relu(x^T): fuse relu into the PSUM->SBUF copy
            h1T = epp.tile([HID, 128], F32, tag="h1T", name="h1T")
            nc.scalar.activation(h1T[:], pT, ACTF.Relu)
            ps2 = psM[:, 320:320 + HID]
            nc.tensor.matmul(out=ps2, lhsT=h1T[:], rhs=W2p[:],
                             start=True, stop=True)
            nc.vector.tensor_copy(t2keep[:, j * HID:(j + 1) * HID], ps2)
            nc.gpsimd.dma_start(t2_sh[j * WIN:j * WIN + wlen, :],
                                t2keep[:wlen, j * HID:(j + 1) * HID])
            if j in q_of_block_end and not no_collectives:
                q = q_of_block_end[j]
                qlo, qhi = rq[q], rq[q + 1]
                ob = qbase[q] * HID
                oe = ob + N_CORES * (qhi - qlo) * HID
                nc.gpsimd.collective_compute(
                    "AllGather", AL.bypass, replica_groups=rg,
                    ins=[t2_sh[qlo:qhi, :]],
                    outs=[t2flat[ob:oe]])

        jb = 0
        while jb < NWIN:
            je = min(jb + JBLOCK, NWIN)
            for qe in sorted(q_of_block_end):
                if jb <= qe < je:
                    je = qe + 1
                    break
            c0, c1 = wstart[jb], wstart[je]
            if c1 > c0:
                wch = wdep.tile([128, maxbt * DSLOT], BF16, tag="wde",
                                name="wch")
                cw = c1 - c0
                nc.sync.dma_start(wch[:, :cw * DSLOT],
                                  d_wde.ap()[:, c0 * DSLOT:c1 * DSLOT])
                HS1, HS2 = DSLOT // 2, DSLOT // 4
                t1 = wdep1.tile([128, maxbt * (DSLOT // 2)], BF16,
                               tag="t1", name="t1")
                with nc.allow_low_precision(reason="deg sums fit bf16"):
                    nc.vector.tensor_tensor(
                        out=t1[:, :cw * HS1].rearrange("p (j s) -> p j s",
                                                       s=HS1),
                        in0=wch[:, :cw * DSLOT].rearrange(
                            "p (j s) -> p j s", s=DSLOT)[:, :, 0:HS1],
                        in1=wch[:, :cw * DSLOT].rearrange(
                            "p (j s) -> p j s", s=DSLOT)[:, :, HS1:DSLOT],
                        op=AL.add)
                    t2t = wdep1.tile([128, maxbt * (DSLOT // 4)], BF16,
                                    tag="t2t", name="t2t")
                    nc.vector.tensor_tensor(
                        out=t2t[:, :cw * HS2].rearrange("p (j s) -> p j s",
                                                        s=HS2),
                        in0=t1[:, :cw * HS1].rearrange("p (j s) -> p j s",
                                                       s=HS1)[:, :, 0:HS2],
                        in1=t1[:, :cw * HS1].rearrange("p (j s) -> p j s",
                                                       s=HS1)[:, :, HS2:HS1],
                        op=AL.add)
                    degs = wdep1.tile([128, maxbt], BF16, tag="degs",
                                     name="degs")
                    nc.vector.tensor_reduce(
                        out=degs[:, :cw].rearrange("p (j s) -> p j s", s=1),
                        in_=t2t[:, :cw * HS2].rearrange("p (j s) -> p j s",
                                                        s=HS2),
                        op=AL.add, axis=mybir.AxisListType.X)
                rsq = wdep1.tile([128, maxbt], F32, tag="rsq",
                                name="rsq")
                nc.scalar.activation(rsq[:, :cw], degs[:, :cw], ACTF.Sqrt)
                nc.vector.reciprocal(rsq[:, :cw], rsq[:, :cw])
                nc.vector.tensor_tensor(out=wprime[:, c0:c1],
                                        in0=rsq[:, :cw],
                                        in1=sb_ewt[:, c0:c1], op=AL.mult)
                nc.vector.tensor_scalar(out=negwp[:, c0:c1],
                                        in0=wprime[:, c0:c1],
                                        scalar1=-1.0, scalar2=None,
                                        op0=AL.mult)

            for j in range(jb, je):
                p1_front(j)
                if j > 0:
                    p1_tail(j - 1)
                if j == NWIN - 1:
                    p1_tail(j)
            jb = je

        # ---- L2: per-(q, block) gathers; window-major chains + SBUF flush ----
        n_pool_done = [0]

        def finish_window(j):
            h2e = epp.tile([128, HID + 1], BF16, tag="h2e", name="h2e")
            nc.vector.scalar_tensor_tensor(
                out=h2e[:, :HID], in0=h2acc[:, j * HID:(j + 1) * HID],
                scalar=dinv[:, j:j + 1],
                in1=c2_b[:], op0=AL.mult, op1=AL.add)
            nc.scalar.activation(h2e[:, :HID], h2e[:, :HID], ACTF.Relu)
            nc.vector.memset(h2e[:, HID:], 1.0)
            ohb = ohp.tile([128, 128], BF16, tag="ohb", name="ohb")
            nc.vector.tensor_scalar(
                out=ohb[:], in0=iota[:],
                scalar1=sb_batch[:, j:j + 1], scalar2=None, op0=AL.is_equal)
            nc.tensor.matmul(out=pool_ps,
                             lhsT=ohb[:],
                             rhs=h2e[:],
                             start=(n_pool_done[0] == 0),
                             stop=(n_pool_done[0] == NWIN - 1),
                             skip_group_check=True)
            n_pool_done[0] += 1

        for gi, (q, jlo, jhi) in enumerate(groups):
                calls = [c for c in gcalls if c[0] == gi]
                if calls:
                    g_lo = calls[0][2]
                    g_hi = calls[-1][3]
                    msgs = msgsp.tile([128, MSGS_TILES * NPAR * HID], FP8,
                                      tag="mG", name="msgs")
                    assert g_hi - g_lo <= MSGS_TILES, (gi, g_hi - g_lo)
                    for (_, r, k_lo, k_hi) in calls:
                        mo = (k_lo - g_lo) * NPAR * HID
                        if no_gather:
                            nc.vector.memset(
                                msgs[:, mo:mo + (k_hi - k_lo) * NPAR * HID],
                                0.0)
                        else:
                            nc.gpsimd.dma_gather(
                                out_ap=msgs[
                                    :, mo:mo + (k_hi - k_lo) * NPAR * HID]
                                .rearrange("p (t h) -> p t h", h=NPAR * HID),
                                in_ap=t2vr[r][:, 0:NPAR * HID],
                                idxs_ap=sb_idx[:, k_lo * 8:k_hi * 8],
                                num_idxs=(k_hi - k_lo) * 128,
                                num_idxs_reg=(k_hi - k_lo) * 128,
                                elem_size=NPAR * HID, elem_step=NPAR * HID,
                                single_packet=False)
                else:
                    msgs, g_lo = None, 0
                for j in range(jlo, jhi):
                    tl = qb_tiles.get((gi, j), [])
                    if not tl and q != 0:
                        continue
                    ch = chainb[j % 2]
                    first = True
                    if q == 0:
                        nc.tensor.matmul(
                            out=ch,
                            lhsT=dgall[:, j * 128:(j + 1) * 128],
                            rhs=t2keep[:, j * HID:(j + 1) * HID],
                            start=True, stop=(len(tl) == 0),
                            skip_group_check=True)
                        first = False
                    for i, (k, col) in enumerate(tl):
                        oh = ohp.tile([128, 128], BF16, tag="oh", name="oh")
                        build_oh(oh[:], col, k, l2_eng(k))
                        mc = (k - g_lo) * NPAR * HID
                        nc.tensor.matmul(out=ch, lhsT=oh[:],
                                         rhs=msgs[:, mc:mc + HID],
                                         start=first,
                                         stop=(i == len(tl) - 1),
                                         skip_group_check=True)
                        first = False
                    hs = h2acc[:, j * HID:(j + 1) * HID]
                    if q == 0:
                        nc.vector.tensor_copy(hs, ch)
                    else:
                        nc.vector.tensor_tensor(out=hs, in0=hs, in1=ch,
                                                op=AL.add)
                    if q == NQ - 1:
                        finish_window(j)

        # ---- pooled partial exchange (AllGather + on-device sum) ----
        pool_sb = epp.tile([128, HID + 1], F32, tag="poolsb", name="pool_sb")
        nc.vector.tensor_copy(pool_sb[:], pool_ps)
        ar_in = dram.tile([128, HID + 1], F32)
        ag_out = dram.tile([N_CORES * 128, HID + 1], F32)
        nc.sync.dma_start(ar_in[:], pool_sb[:])
        allsb = epp.tile([128, N_CORES * (HID + 1)], F32, tag="allsb",
                         name="allsb")
        if no_collectives:
            for c in range(N_CORES):
                nc.sync.dma_start(ag_out[c * 128:(c + 1) * 128, :], ar_in[:])
        else:
            nc.gpsimd.collective_compute(
                "AllGather", AL.bypass, replica_groups=rg,
                ins=[ar_in.opt()], outs=[ag_out.opt()])
        nc.sync.dma_start(
            allsb[:].rearrange("p (c h) -> p c h", c=N_CORES),
            ag_out[:].rearrange("(c p) h -> p c h", c=N_CORES))
        sums = epp.tile([128, HID + 1], F32, tag="sums", name="sums")
        nc.vector.tensor_reduce(
            out=sums[:].rearrange("p (h s) -> p h s", s=1),
            in_=allsb[:].rearrange("p (c h) -> p h c", c=N_CORES),
            op=AL.add, axis=mybir.AxisListType.X)

        cntc = epp.tile([128, 1], F32, tag="cnt", name="cntc")
        nc.vector.tensor_scalar(out=cntc[:], in0=sums[:, HID:HID + 1],
                                scalar1=1.0, scalar2=None, op0=AL.max)
        rc = epp.tile([128, 1], F32, tag="rc", name="rc")
        nc.vector.reciprocal(rc[:], cntc[:])
        pooled = epp.tile([128, HID], F32, tag="pooled", name="pooled")
        nc.vector.tensor_scalar(out=pooled[:], in0=sums[:, :HID],
                                scalar1=rc[:, :1], scalar2=None, op0=AL.mult)
        pT2 = psM2[0:HID, 192:320]
        nc.tensor.transpose(out=pT2, in_=pooled[:], identity=ident[:])
        pooledT = epp.tile([HID, 128], F32, tag="pooledT", name="pooledT")
        nc.vector.tensor_copy(pooledT[:], pT2)
        zps = psM2[0:H2, 0:128]
        nc.tensor.matmul(out=zps, lhsT=sb_lin1W[:], rhs=pooledT[:],
                         start=True, stop=True)
        zT = epp.tile([H2, 128], F32, tag="zT", name="zT")
        nc.scalar.activation(zT[:], zps, ACTF.Relu, bias=sb_lin1b[:, :1])
        ops = psM2[0:1, 320:448]
        nc.tensor.matmul(out=ops, lhsT=sb_lin2W[:], rhs=zT[:],
                         start=True, stop=True)
        outsb = epp.tile([1, 128], F32, tag="outsb", name="outsb")
        nc.vector.tensor_scalar(out=outsb[:], in0=ops,
                                scalar1=sb_lin2b[:, :1], scalar2=None,
                                op0=AL.add)
        nc.sync.dma_start(d_out.ap(), outsb[:])

    nc.compile()
    return nc


_CACHE = {}


def kernel(**inputs) -> np.ndarray:
    in_maps, meta = _prep_inputs(inputs)
    key = (meta["N"], meta["T"], meta["DSLOT"])
    if key not in _CACHE:
        _CACHE[key] = _build_nc(meta)
    nc = _CACHE[key]
    res = run_bass_kernel_spmd(nc, in_maps, core_ids=list(range(N_CORES)))
    out = np.asarray(res.results[0]["out"], np.float32).reshape(-1)
    return out[:meta["G"]].copy()


# revision 3
# speedup vs baseline: 1.1199x; 1.1199x over previous
"""BrainGCN on 8 Trainium2 NeuronCores (Bass/Tile, SPMD) — v2.

kernel(**inputs) takes FULL unsharded inputs, returns the full (G,) output.

Sharding: N nodes in 8 contiguous shards (SH=N/8); edges live on the core
owning their dst node, grouped into 128-node dst windows.  Tiles of 128
edges are classified by (window j, src%NPAR parity r, src-chunk q) where q
indexes NQ slices of every shard (for AllGather pipelining); per-(j,r) tile
counts are equalized across cores so one SPMD program serves all 8.

Unified one-hot: both layers use per-edge coefficient w'_e = w_e *
rsqrt(deg[src]).  The t2 table is h1@W2p WITHOUT the dst dinv fold, so a
single one-hot (iota==dloc)*w' drives both the L1 aggregation (rhs=onehot,
lhsT=host-pregathered x[src] rows) and the L2 aggregation (lhsT=onehot,
rhs=per-edge gathered t2 rows).  One-hots are built twice (P1 and L2) on a
DVE/Pool/ACT mix; self-loops enter via diag(dinv) matmuls.

w' comes from host-pregathered per-edge degree-slot rows (wde) reduced on
device (bf16) + Rsqrt; all FLOPs stay on device (host only permutes/pads
input data).

L2 messages: t2 stored fp8e4 [N,64] (64B rows); SWDGE bulk dma_gather with
256B elems (4 rows), NPAR=4 src-parity classes picking the row via shifted
table views; idx16 = src//4.  The AllGather runs in NQ=4 window-aligned
chunks, each fired as soon as every core has written that slice of its t2
shard, so gathers for chunk q overlap P1 compute of later chunks.

Scatter-add = one-hot matmuls into 49 concurrently-open PSUM accumulation
chains (one per dst window, skip_group_check).  Pooling = one-hot(batch)
matmul with a ones column; partial [G, H+1] pools are AllGathered (cheaper
than AllReduce) and summed on-device; the tiny MLP head is replicated.
"""
import math
from contextlib import ExitStack

import numpy as np
import ml_dtypes

import concourse.bass as bass
import concourse.bacc as bacc
import concourse.tile as tile
import concourse.mybir as mybir
from concourse import library_config
from concourse.bass_utils import run_bass_kernel_spmd

F32 = mybir.dt.float32
BF16 = mybir.dt.bfloat16
FP8 = mybir.dt.float8e4
I16 = mybir.dt.int16
AL = mybir.AluOpType
ACTF = mybir.ActivationFunctionType
BF = ml_dtypes.bfloat16
F8 = ml_dtypes.float8_e4m3

N_CORES = 8
WIN = 128
EPS = 1e-5
NQ = 4            # AllGather chunks (window-aligned slices of each shard)
NPAR = 4          # src parity classes (fp8 rows: 256B elem = 4 rows)
CALL_TILES = 32   # tiles per dma_gather call (legacy, unused)
MSGS_TILES = 64   # tiles buffered per (q, group) msgs buffer
JBLOCK = 5        # windows per wde streaming chunk


def _prep_inputs(inputs: dict):
    x = np.asarray(inputs["x"], np.float32)
    ei = np.asarray(inputs["edge_index"])
    ew = np.asarray(inputs["edge_weight"], np.float32)
    batch = np.asarray(inputs["batch"]).astype(np.int64)
    N, IN_DIM = x.shape
    HID = np.asarray(inputs["W1"]).shape[1]
    assert N % N_CORES == 0
    SH = N // N_CORES
    NWIN = math.ceil(SH / WIN)

    # window-aligned q boundaries (in windows, then node rows of each shard).
    # Front-loaded: a tile of 128 edges straddles into the next chunk, so
    # early chunks get extra windows to keep per-q gather volumes balanced.
    wq = [0, 15, 28, 41, NWIN] if NWIN == 49 else \
        [round(q * NWIN / NQ) for q in range(NQ + 1)]
    rq = [min(w * WIN, SH) for w in wq]

    # degree rows incl self-loop weight 1 (self-loops excluded from edges)
    dstA = np.concatenate([np.asarray(ei[1]), np.arange(N)]).astype(np.int64)
    wA = np.concatenate([ew, np.ones(N, np.float32)]).astype(np.float32)
    orderA = np.argsort(dstA, kind="stable")
    dsA, wsA = dstA[orderA], wA[orderA]
    countsA = np.bincount(dstA, minlength=N)
    DSLOT = int(countsA.max())
    rowptrA = np.zeros(N + 1, np.int64)
    np.cumsum(countsA, out=rowptrA[1:])
    wdeg_full = np.zeros((N, DSLOT), np.float32)
    slotA = np.arange(len(dsA)) - rowptrA[dsA]
    wdeg_full[dsA, slotA] = wsA

    src = np.asarray(ei[0]).astype(np.int64)
    dst = np.asarray(ei[1]).astype(np.int64)
    w = ew.astype(np.float32)
    order = np.argsort(dst, kind="stable")
    ds, ss, ws = dst[order], src[order], w[order]
    qs = np.searchsorted(rq, ss % SH, side="right") - 1  # src q-chunk
    # chunk-major table layout [q][core][local row][h]: each chunk's
    # AllGather output is one contiguous block (BIR requires contiguity).
    Bq = [rq[q + 1] - rq[q] for q in range(NQ)]
    qbase = np.concatenate([[0], np.cumsum([N_CORES * b for b in Bq])])
    c_src = ss // SH
    i_src = ss % SH
    rows = (qbase[qs] + c_src * np.array(Bq)[qs]
            + (i_src - np.array(rq)[qs])).astype(np.int64)
    rs = rows % NPAR
    counts = np.bincount(dst, minlength=N)
    rowptr = np.zeros(N + 1, np.int64)
    np.cumsum(counts, out=rowptr[1:])

    # per-core (window, parity) segments, sorted by q inside
    seg = {}
    cnt = np.zeros((N_CORES, NWIN, NPAR), np.int64)
    for c in range(N_CORES):
        for j in range(NWIN):
            lo = c * SH + j * WIN
            hi = min(c * SH + (j + 1) * WIN, (c + 1) * SH)
            a, b = int(rowptr[lo]), int(rowptr[hi])
            sl = slice(a, b)
            for r in range(NPAR):
                idxr = np.nonzero(rs[sl] == r)[0]
                idxr = idxr[np.argsort(qs[sl][idxr], kind="stable")]
                seg[(c, j, r)] = (lo, idxr)
                cnt[c, j, r] = len(idxr)

    ntile = np.zeros((NWIN, NPAR), np.int64)
    for j in range(NWIN):
        for r in range(NPAR):
            ntile[j, r] = -(-int(cnt[:, j, r].max()) // 128)
    T = int(ntile.sum())

    # tile q-class = max over cores of (q of last edge in tile).  Edges are
    # placed partial-tile-FIRST (pads at the very front), so the lone
    # partial tile of each cell carries the lowest-q edges and classifies
    # early instead of inflating the last chunk.
    tclass = {}
    for j in range(NWIN):
        for r in range(NPAR):
            nt = int(ntile[j, r])
            cap = nt * 128
            for t in range(nt):
                qmax = 0
                for c in range(N_CORES):
                    lo, idxr = seg[(c, j, r)]
                    n = len(idxr)
                    off = cap - n
                    e1 = (t + 1) * 128 - 1 - off
                    if e1 < 0:
                        continue
                    e1 = min(e1, n - 1)
                    a = int(rowptr[lo])
                    qmax = max(qmax, int(qs[a + idxr[e1]]))
                tclass[(j, r, t)] = qmax

    # window-major column order (for edloc/ewt/xg/wde/wprime)
    colW = {}
    c_acc = 0
    for j in range(NWIN):
        for r in range(NPAR):
            for t in range(int(ntile[j, r])):
                colW[(j, r, t)] = c_acc
                c_acc += 1
    assert c_acc == T

    # gather order: (q, window-group, r, j, t).  One dma_gather call per
    # (q, group, r); within a (q, group) the matmul sweep is window-major so
    # each PSUM bank has at most one open accumulation chain (hardware
    # corrupts interleaved chains within a bank).  Groups are greedy runs of
    # windows holding at most MSGS_TILES q-class tiles.
    qtiles_per_win = np.zeros((NQ, NWIN), np.int64)
    for (j, r, t), q in tclass.items():
        qtiles_per_win[q, j] += 1
    groups = []            # (q, jlo, jhi)
    for q in range(NQ):
        jlo = 0
        while jlo < NWIN:
            jhi, tot = jlo, 0
            while jhi < NWIN and tot + qtiles_per_win[q, jhi] <= MSGS_TILES:
                tot += qtiles_per_win[q, jhi]
                jhi += 1
            assert jhi > jlo
            groups.append((q, jlo, jhi))
            jlo = jhi
    gorder = []
    gcalls = []            # (gi, r, k_lo, k_hi)
    qb_tiles = {}          # (gi, j) -> [(k, colW)]
    for gi, (q, jlo, jhi) in enumerate(groups):
        for r in range(NPAR):
            k_lo = len(gorder)
            for j in range(jlo, jhi):
                for t in range(int(ntile[j, r])):
                    if tclass[(j, r, t)] == q:
                        gorder.append((j, r, t))
                        qb_tiles.setdefault((gi, j), []).append(
                            (len(gorder) - 1, colW[(j, r, t)]))
            if len(gorder) > k_lo:
                gcalls.append((gi, r, k_lo, len(gorder)))
    assert len(gorder) == T
    kmap = {jrt: k for k, jrt in enumerate(gorder)}

    win_tiles = [[] for _ in range(NWIN)]
    for j in range(NWIN):
        for r in range(NPAR):
            for t in range(int(ntile[j, r])):
                win_tiles[j].append((colW[(j, r, t)], kmap[(j, r, t)]))
    tile_info = [(j, colW[(j, r, t)]) for (j, r, t) in gorder]

    # --- per-core arrays ---
    edloc = np.full((N_CORES, 128, T), 999.0, np.float32)
    ewt = np.zeros((N_CORES, 128, T), np.float32)
    xg = np.zeros((N_CORES, 128, T * IN_DIM), BF)
    wde = np.zeros((N_CORES, 128, T * DSLOT), BF)
    idx16 = np.zeros((N_CORES, 16, T * 8), np.int16)

    for c in range(N_CORES):
        for j in range(NWIN):
            for r in range(NPAR):
                lo, idxr = seg[(c, j, r)]
                a = int(rowptr[lo])
                n = len(idxr)
                nt = int(ntile[j, r])
                cap = nt * 128
                off = cap - n
                s_pad = np.zeros(cap, np.int64)
                row_pad = np.zeros(cap, np.int64)
                d_pad = np.full(cap, 999.0, np.float32)
                w_pad = np.zeros(cap, np.float32)
                s_pad[off:] = ss[a + idxr]
                row_pad[off:] = rows[a + idxr]
                d_pad[off:] = (ds[a + idxr] - lo).astype(np.float32)
                w_pad[off:] = ws[a + idxr]
                padm = d_pad > 900.0
                for t in range(nt):
                    cw = colW[(j, r, t)]
                    k = kmap[(j, r, t)]
                    blk = slice(t * 128, (t + 1) * 128)
                    pm = padm[blk]
                    edloc[c, :, cw] = d_pad[blk]
                    ewt[c, :, cw] = w_pad[blk]
                    xs = x[s_pad[blk]].astype(np.float32)
                    xs[pm] = 0.0
                    xg[c, :, cw * IN_DIM:(cw + 1) * IN_DIM] = xs.astype(BF)
                    wd = wdeg_full[s_pad[blk]].copy()
                    wd[pm] = 0.0
                    wd[pm, 0] = 1.0
                    wde[c, :, cw * DSLOT:(cw + 1) * DSLOT] = wd.astype(BF)
                    iv = (row_pad[blk] // NPAR).astype(np.int16)
                    iv[pm] = 0
                    iv2 = iv.reshape(8, 16)  # [p//16, p%16]
                    idx16[c, :, k * 8:(k + 1) * 8] = iv2.T

    PADN = NWIN * WIN

    def win_major(a2d, dt=np.float32):
        S = a2d.shape[1]
        assert a2d.shape[0] == PADN
        return np.ascontiguousarray(
            a2d.reshape(NWIN, WIN, S).transpose(1, 0, 2)
            .reshape(WIN, NWIN * S)).astype(dt)

    iota = np.tile(np.arange(128, dtype=np.float32), (128, 1))
    ident = np.eye(128, dtype=np.float32)

    in_maps = []
    for c in range(N_CORES):
        wc = np.zeros((PADN, DSLOT), np.float32)
        wc[:SH] = wdeg_full[c * SH:(c + 1) * SH]
        wc[SH:, 0] = 1.0
        bv = np.full((PADN, 1), 999.0, np.float32)
        bv[:SH, 0] = batch[c * SH:(c + 1) * SH].astype(np.float32)
        xo = np.zeros((PADN, IN_DIM), np.float32)
        xo[:SH] = x[c * SH:(c + 1) * SH]
        in_maps.append({
            "edloc": edloc[c], "ewt": ewt[c].astype(BF),
            "xg": xg[c], "wde": wde[c],
            "idx16": np.tile(idx16[c], (8, 1)),
            "xnm": win_major(xo, BF),
            "wdeg": win_major(wc, BF), "batchv": win_major(bv),
            "iota": iota.astype(BF), "ident": ident,
            "W1": np.asarray(inputs["W1"], np.float32),
            "W2": np.asarray(inputs["W2"], np.float32),
            "g1": np.asarray(inputs["bn1_gamma"], np.float32).reshape(1, HID),
            "be1": np.asarray(inputs["bn1_beta"], np.float32).reshape(1, HID),
            "m1": np.asarray(inputs["bn1_mean"], np.float32).reshape(1, HID),
            "v1": np.asarray(inputs["bn1_var"], np.float32).reshape(1, HID),
            "b1": np.asarray(inputs["b1"], np.float32).reshape(1, HID),
            "g2": np.asarray(inputs["bn2_gamma"], np.float32).reshape(1, HID),
            "be2": np.asarray(inputs["bn2_beta"], np.float32).reshape(1, HID),
            "m2": np.asarray(inputs["bn2_mean"], np.float32).reshape(1, HID),
            "v2": np.asarray(inputs["bn2_var"], np.float32).reshape(1, HID),
            "b2": np.asarray(inputs["b2"], np.float32).reshape(1, HID),
            "lin1W": np.asarray(inputs["lin1_W"], np.float32),
            "lin1b": np.asarray(inputs["lin1_b"], np.float32).reshape(-1, 1),
            "lin2W": np.asarray(inputs["lin2_W"], np.float32),
            "lin2b": np.asarray(inputs["lin2_b"], np.float32).reshape(1, 1),
        })

    meta = dict(N=N, G=128, IN_DIM=IN_DIM, HID=HID, SH=SH, NWIN=NWIN,
                DSLOT=DSLOT, T=T, wq=wq, rq=rq,
                qbase=[int(x) for x in qbase], Bq=Bq,
                win_tiles=win_tiles, tile_info=tile_info,
                gcalls=gcalls, qb_tiles=qb_tiles, groups=groups)
    return in_maps, meta


def _build_nc(meta, no_collectives=False, no_gather=False):
    N, IN_DIM, HID = meta["N"], meta["IN_DIM"], meta["HID"]
    SH, NWIN, DSLOT, T = meta["SH"], meta["NWIN"], meta["DSLOT"], meta["T"]
    wq, rq, qbase = meta["wq"], meta["rq"], meta["qbase"]
    win_tiles = meta["win_tiles"]
    tile_info = meta["tile_info"]
    gcalls, qb_tiles, groups = meta["gcalls"], meta["qb_tiles"], meta["groups"]
    H2 = HID // 2

    nc = bacc.Bacc("TRN2", target_bir_lowering=False, debug=False,
                   num_devices=N_CORES)
    d_edloc = nc.dram_tensor("edloc", [128, T], F32, kind="ExternalInput")
    d_ewt = nc.dram_tensor("ewt", [128, T], BF16, kind="ExternalInput")
    d_xg = nc.dram_tensor("xg", [128, T * IN_DIM], BF16, kind="ExternalInput")
    d_wde = nc.dram_tensor("wde", [128, T * DSLOT], BF16, kind="ExternalInput")
    d_idx = nc.dram_tensor("idx16", [128, T * 8], I16, kind="ExternalInput")
    d_xnm = nc.dram_tensor("xnm", [128, NWIN * IN_DIM], BF16,
                           kind="ExternalInput")
    d_wdeg = nc.dram_tensor("wdeg", [128, NWIN * DSLOT], BF16,
                            kind="ExternalInput")
    d_batch = nc.dram_tensor("batchv", [128, NWIN], F32, kind="ExternalInput")
    d_iota = nc.dram_tensor("iota", [128, 128], BF16, kind="ExternalInput")
    d_ident = nc.dram_tensor("ident", [128, 128], F32, kind="ExternalInput")
    d_W1 = nc.dram_tensor("W1", [IN_DIM, HID], F32, kind="ExternalInput")
    d_W2 = nc.dram_tensor("W2", [HID, HID], F32, kind="ExternalInput")
    bn_names = ["g1", "be1", "m1", "v1", "b1", "g2", "be2", "m2", "v2", "b2"]
    d_bn = {k: nc.dram_tensor(k, [1, HID], F32, kind="ExternalInput")
            for k in bn_names}
    d_lin1W = nc.dram_tensor("lin1W", [HID, H2], F32, kind="ExternalInput")
    d_lin1b = nc.dram_tensor("lin1b", [H2, 1], F32, kind="ExternalInput")
    d_lin2W = nc.dram_tensor("lin2W", [H2, 1], F32, kind="ExternalInput")
    d_lin2b = nc.dram_tensor("lin2b", [1, 1], F32, kind="ExternalInput")
    d_out = nc.dram_tensor("out", [1, 128], F32, kind="ExternalOutput")

    rg = [list(range(N_CORES))]

    with tile.TileContext(nc) as tc, ExitStack() as ctx:
        constp = ctx.enter_context(tc.tile_pool(name="const", bufs=1))
        metap = ctx.enter_context(tc.tile_pool(name="meta", bufs=1))
        wdep = ctx.enter_context(tc.tile_pool(name="wdep", bufs=2))
        msgsp = ctx.enter_context(tc.tile_pool(name="msgs", bufs=2))
        wdep1 = ctx.enter_context(tc.tile_pool(name="wdep1", bufs=1))
        ohp = ctx.enter_context(tc.tile_pool(name="oh", bufs=16))
        epp = ctx.enter_context(tc.tile_pool(name="ep", bufs=4))
        vecp = ctx.enter_context(tc.tile_pool(name="vec", bufs=1))
        psA = ctx.enter_context(tc.tile_pool(name="psA", bufs=1,
                                              space="PSUM"))
        dram = ctx.enter_context(tc.tile_pool(name="dram", bufs=1,
                                              space="DRAM"))

        nc.gpsimd.load_library(library_config.mlp)

        # ---- constants and small inputs ----
        iota = constp.tile([128, 128], BF16)
        nc.sync.dma_start(iota[:], d_iota.ap())
        ident = constp.tile([128, 128], F32)
        nc.sync.dma_start(ident[:], d_ident.ap())
        ones1 = constp.tile([1, 128], F32)
        nc.vector.memset(ones1[:], 1.0)

        sb_edloc = metap.tile([128, T], F32)
        nc.sync.dma_start(sb_edloc[:], d_edloc.ap())
        sb_ewt = metap.tile([128, T], BF16)
        nc.sync.dma_start(sb_ewt[:], d_ewt.ap())
        sb_xg = metap.tile([128, T * IN_DIM], BF16)
        nc.sync.dma_start(sb_xg[:], d_xg.ap())
        sb_idx = metap.tile([128, T * 8], I16)
        nc.sync.dma_start(sb_idx[:], d_idx.ap())
        sb_xnm = metap.tile([128, NWIN * IN_DIM], BF16)
        nc.sync.dma_start(sb_xnm[:], d_xnm.ap())
        sb_wdeg = metap.tile([128, NWIN * DSLOT], BF16)
        nc.sync.dma_start(sb_wdeg[:], d_wdeg.ap())
        sb_batch = metap.tile([128, NWIN], F32)
        nc.sync.dma_start(sb_batch[:], d_batch.ap())
        sb_W1 = constp.tile([IN_DIM, HID], F32)
        nc.sync.dma_start(sb_W1[:], d_W1.ap())
        sb_W2 = constp.tile([HID, HID], F32)
        nc.sync.dma_start(sb_W2[:], d_W2.ap())
        sb_bn = {}
        for k in bn_names:
            sb_bn[k] = vecp.tile([1, HID], F32, tag=k, name="sb_" + k)
            nc.sync.dma_start(sb_bn[k][:], d_bn[k].ap())
        sb_lin1W = constp.tile([HID, H2], F32)
        sb_lin1b = constp.tile([H2, 1], F32)
        sb_lin2W = constp.tile([H2, 1], F32)
        sb_lin2b = constp.tile([1, 1], F32)
        nc.sync.dma_start(sb_lin1W[:], d_lin1W.ap())
        nc.sync.dma_start(sb_lin1b[:], d_lin1b.ap())
        nc.sync.dma_start(sb_lin2W[:], d_lin2W.ap())
        nc.sync.dma_start(sb_lin2b[:], d_lin2b.ap())

        # ---- BN folds ----
        def bn_fold(g, be, m, v, b):
            bns = vecp.tile([1, HID], F32, tag="bns" + g, name="bns" + g)
            nc.vector.tensor_scalar(out=bns[:], in0=sb_bn[v][:], scalar1=EPS,
                                    scalar2=None, op0=AL.add)
            nc.scalar.activation(bns[:], bns[:], ACTF.Sqrt)
            nc.vector.reciprocal(bns[:], bns[:])
            nc.vector.tensor_tensor(out=bns[:], in0=bns[:], in1=sb_bn[g][:],
                                    op=AL.mult)
            cc = vecp.tile([1, HID], F32, tag="c" + g, name="c" + g)
            nc.vector.tensor_tensor(out=cc[:], in0=sb_bn[b][:],
                                    in1=sb_bn[m][:], op=AL.subtract)
            nc.vector.tensor_tensor(out=cc[:], in0=cc[:], in1=bns[:],
                                    op=AL.mult)
            nc.vector.tensor_tensor(out=cc[:], in0=cc[:], in1=sb_bn[be][:],
                                    op=AL.add)
            return bns, cc

        bns1, c1v = bn_fold("g1", "be1", "m1", "v1", "b1")
        bns2, c2v = bn_fold("g2", "be2", "m2", "v2", "b2")

        # PSUM banks: two rotating L2 chain banks (one open accumulation
        # chain per bank at a time — interleaved chains within a bank are
        # corrupted by hardware), a pooling bank, and two P1 transient banks.
        chainb = [psA.tile([128, HID], F32, tag=f"ch{i}", name=f"ch{i}")[:]
                  for i in range(2)]
        poolbk = psA.tile([128, HID + 1], F32, tag="poolb", name="poolb")
        pool_ps = poolbk[:]
        psM1 = psA.tile([128, 512], F32, tag="psM1", name="psM1")
        psM2 = psA.tile([128, 512], F32, tag="psM2", name="psM2")

        def bcast128(vec, tag):
            ps = psM2[:, 128:128 + HID]
            nc.tensor.matmul(out=ps, lhsT=ones1[:], rhs=vec[:],
                             start=True, stop=True)
            sb = constp.tile([128, HID], F32, tag=tag, name="sb" + tag)
            nc.vector.tensor_copy(sb[:], ps)
            return sb

        c1_b = bcast128(c1v, "c1b")
        c2_b = bcast128(c2v, "c2b")

        def wfold(sb_W, bns, parts, tag):
            one_r = constp.tile([1, parts], F32, tag="oner" + tag,
                                name="oner" + tag)
            nc.vector.memset(one_r[:], 1.0)
            ps = psM2[0:parts, 128:128 + HID]
            nc.tensor.matmul(out=ps, lhsT=one_r[:], rhs=bns[:],
                             start=True, stop=True)
            wp = constp.tile([parts, HID], F32, tag="wp" + tag,
                             name="wp" + tag)
            nc.vector.tensor_tensor(out=wp[:], in0=sb_W[:], in1=ps[:],
                                    op=AL.mult)
            return wp

        W1p = wfold(sb_W1, bns1, IN_DIM, "1")
        W2p = wfold(sb_W2, bns2, HID, "2")

        # ---- dst dinv + persisted diag / pooling one-hots ----
        dinv = constp.tile([128, NWIN], F32)
        with nc.allow_low_precision(reason="deg sums fit bf16"):
            degd = epp.tile([128, NWIN], BF16, tag="degd", name="degd")
            nc.vector.tensor_reduce(
                out=degd[:].rearrange("p (j s) -> p j s", s=1),
                in_=sb_wdeg[:].rearrange("p (j s) -> p j s", s=DSLOT),
                op=AL.add, axis=mybir.AxisListType.X)
        nc.scalar.activation(dinv[:], degd[:], ACTF.Sqrt)
        nc.vector.reciprocal(dinv[:], dinv[:])

        dgall = constp.tile([128, NWIN * 128], BF16)

        # ---- w' = ewt * rsqrt(deg[src]) (streamed with P1 below) ----
        wprime = metap.tile([128, T], F32)
        negdloc = metap.tile([128, T], F32)
        negwp = metap.tile([128, T], F32)
        nc.vector.tensor_scalar(out=negdloc[:], in0=sb_edloc[:],
                                scalar1=-1.0, scalar2=None, op0=AL.mult)

        # t2 table (fp8) in DRAM
        t2_sh = dram.tile([SH, HID], FP8)
        t2_full = dram.tile([N + NPAR, HID], FP8)
        t2flat = t2_full[:].rearrange("n h -> (n h)")
        NROW4 = N // NPAR
        t2vr = [t2flat[r * HID: r * HID + NROW4 * NPAR * HID]
                .rearrange("(m k) -> m k", k=NPAR * HID)
                for r in range(NPAR)]
        zrow = constp.tile([NPAR, HID], FP8)
        nc.vector.memset(zrow[:], 0.0)
        nc.sync.dma_start(t2_full[N:N + NPAR, :], zrow[:])

        t2keep = metap.tile([128, NWIN * HID], BF16)
        h2acc = metap.tile([128, NWIN * HID], F32)

        # one-hot build engine schedule.  Pool builds only in early P1
        # windows (its in-order queue must reach the L2 gathers before AG0
        # completes); ACT carries a slice everywhere; DVE the rest.
        def build_oh(dst_ap, col, k, eng):
            if eng == "A":
                tt = ohp.tile([128, 128], BF16, tag="att", name="att")
                nc.scalar.activation(tt[:], iota[:], ACTF.Abs,
                                     bias=negdloc[:, col:col + 1])
                nc.scalar.activation(dst_ap, tt[:], ACTF.Relu,
                                     bias=wprime[:, col:col + 1],
                                     scale=negwp[:, col:col + 1])
            elif eng == "P":
                nc.gpsimd.tensor_scalar(
                    out=dst_ap, in0=iota[:],
                    scalar1=sb_edloc[:, col:col + 1],
                    scalar2=wprime[:, col:col + 1],
                    op0=AL.is_equal, op1=AL.mult)
            else:
                nc.vector.tensor_scalar(
                    out=dst_ap, in0=iota[:],
                    scalar1=sb_edloc[:, col:col + 1],
                    scalar2=wprime[:, col:col + 1],
                    op0=AL.is_equal, op1=AL.mult)

        def p1_eng(j, k):
            return "A" if k % 5 == 1 else "D"

        def l2_eng(k):
            return "A" if k % 5 == 1 else "D"

        # ---- P1: per-block wde streaming, per-window L1 + t2, chunked AG ----
        # window-major columns: window j covers [wstart[j], wstart[j+1])
        wstart = [0] * (NWIN + 1)
        for j in range(NWIN):
            wstart[j + 1] = wstart[j] + len(win_tiles[j])
        maxbt = max(wstart[min(j + JBLOCK, NWIN)] - wstart[j]
                    for j in range(NWIN))

        q_of_block_end = {}
        for q in range(NQ):
            q_of_block_end[wq[q + 1] - 1] = q

        def p1_front(j):
            # diag one-hot JIT, then the L1 accumulation chain
            nc.vector.tensor_scalar(
                out=dgall[:, j * 128:(j + 1) * 128], in0=ident[:],
                scalar1=dinv[:, j:j + 1], scalar2=None, op0=AL.mult)
            psM = psM1 if j % 2 == 0 else psM2
            acc5 = psM[0:IN_DIM, 0:128]
            first = True
            for (col, k) in win_tiles[j]:
                oh = ohp.tile([128, 128], BF16, tag="oh", name="oh")
                build_oh(oh[:], col, k, p1_eng(j, k))
                nc.tensor.matmul(
                    out=acc5,
                    lhsT=sb_xg[:, col * IN_DIM:(col + 1) * IN_DIM],
                    rhs=oh[:], start=first, stop=False)
                first = False
            nc.tensor.matmul(
                out=acc5,
                lhsT=sb_xnm[:, j * IN_DIM:(j + 1) * IN_DIM],
                rhs=dgall[:, j * 128:(j + 1) * 128],
                start=first, stop=True)

        def p1_tail(j):
            wlen = min(WIN, SH - j * WIN)
            psM = psM1 if j % 2 == 0 else psM2
            acc5 = psM[0:IN_DIM, 0:128]
            agg5 = epp.tile([IN_DIM, 128], F32, tag="agg5", name="agg5")
            nc.scalar.activation(agg5[:], acc5, ACTF.Copy)
            ps1 = psM[:, 128:128 + HID]
            nc.tensor.matmul(out=ps1, lhsT=agg5[:], rhs=W1p[:],
                             start=True, stop=True)
            h1 = epp.tile([128, HID], F32, tag="h1", name="h1")
            nc.vector.scalar_tensor_tensor(
                out=h1[:], in0=ps1, scalar=dinv[:, j:j + 1],
                in1=c1_b[:], op0=AL.mult, op1=AL.add)
            pT = psM[0:HID, 192:320]
            nc.tensor.transpose(out=pT, in_=h1[:], identity=ident[:])
            # relu(x)^T == relu(x^T): fuse relu into the PSUM->SBUF copy
            h1T = epp.tile([HID, 128], F32, tag="h1T", name="h1T")
            nc.scalar.activation(h1T[:], pT, ACTF.Relu)
            ps2 = psM[:, 320:320 + HID]
            nc.tensor.matmul(out=ps2, lhsT=h1T[:], rhs=W2p[:],
                             start=True, stop=True)
            nc.vector.tensor_copy(t2keep[:, j * HID:(j + 1) * HID], ps2)
            nc.gpsimd.dma_start(t2_sh[j * WIN:j * WIN + wlen, :],
                                t2keep[:wlen, j * HID:(j + 1) * HID])
            if j in q_of_block_end and not no_collectives:
                q = q_of_block_end[j]
                qlo, qhi = rq[q], rq[q + 1]
                ob = qbase[q] * HID
                oe = ob + N_CORES * (qhi - qlo) * HID
                nc.gpsimd.collective_compute(
                    "AllGather", AL.bypass, replica_groups=rg,
                    ins=[t2_sh[qlo:qhi, :]],
                    outs=[t2flat[ob:oe]])

        jb = 0
        while jb < NWIN:
            je = min(jb + JBLOCK, NWIN)
            for qe in sorted(q_of_block_end):
                if jb <= qe < je:
                    je = qe + 1
                    break
            c0, c1 = wstart[jb], wstart[je]
            if c1 > c0:
                wch = wdep.tile([128, maxbt * DSLOT], BF16, tag="wde",
                                name="wch")
                cw = c1 - c0
                nc.sync.dma_start(wch[:, :cw * DSLOT],
                                  d_wde.ap()[:, c0 * DSLOT:c1 * DSLOT])
                HS1, HS2 = DSLOT // 2, DSLOT // 4
                t1 = wdep1.tile([128, maxbt * (DSLOT // 2)], BF16,
                               tag="t1", name="t1")
                with nc.allow_low_precision(reason="deg sums fit bf16"):
                    nc.vector.tensor_tensor(
                        out=t1[:, :cw * HS1].rearrange("p (j s) -> p j s",
                                                       s=HS1),
                        in0=wch[:, :cw * DSLOT].rearrange(
                            "p (j s) -> p j s", s=DSLOT)[:, :, 0:HS1],
                        in1=wch[:, :cw * DSLOT].rearrange(
                            "p (j s) -> p j s", s=DSLOT)[:, :, HS1:DSLOT],
                        op=AL.add)
                    t2t = wdep1.tile([128, maxbt * (DSLOT // 4)], BF16,
                                    tag="t2t", name="t2t")
                    nc.vector.tensor_tensor(
                        out=t2t[:, :cw * HS2].rearrange("p (j s) -> p j s",
                                                        s=HS2),
                        in0=t1[:, :cw * HS1].rearrange("p (j s) -> p j s",
                                                       s=HS1)[:, :, 0:HS2],
                        in1=t1[:, :cw * HS1].rearrange("p (j s) -> p j s",
                                                       s=HS1)[:, :, HS2:HS1],
                        op=AL.add)
                    degs = wdep1.tile([128, maxbt], BF16, tag="degs",
                                     name="degs")
                    nc.vector.tensor_reduce(
                        out=degs[:, :cw].rearrange("p (j s) -> p j s", s=1),
                        in_=t2t[:, :cw * HS2].rearrange("p (j s) -> p j s",
                                                        s=HS2),
                        op=AL.add, axis=mybir.AxisListType.X)
                rsq = wdep1.tile([128, maxbt], F32, tag="rsq",
                                name="rsq")
                nc.scalar.activation(rsq[:, :cw], degs[:, :cw], ACTF.Sqrt)
                nc.vector.reciprocal(rsq[:, :cw], rsq[:, :cw])
                nc.vector.tensor_tensor(out=wprime[:, c0:c1],
                                        in0=rsq[:, :cw],
                                        in1=sb_ewt[:, c0:c1], op=AL.mult)
                nc.vector.tensor_scalar(out=negwp[:, c0:c1],
                                        in0=wprime[:, c0:c1],
                                        scalar1=-1.0, scalar2=None,
                                        op0=AL.mult)

            for j in range(jb, je):
                p1_front(j)
                if j > 0:
                    p1_tail(j - 1)
                if j == NWIN - 1:
                    p1_tail(j)
            jb = je

        # ---- L2: per-(q, block) gathers; window-major chains + SBUF flush ----
        n_pool_done = [0]

        def finish_window(j):
            h2e = epp.tile([128, HID + 1], BF16, tag="h2e", name="h2e")
            nc.vector.scalar_tensor_tensor(
                out=h2e[:, :HID], in0=h2acc[:, j * HID:(j + 1) * HID],
                scalar=dinv[:, j:j + 1],
                in1=c2_b[:], op0=AL.mult, op1=AL.add)
            nc.scalar.activation(h2e[:, :HID], h2e[:, :HID], ACTF.Relu)
            nc.vector.memset(h2e[:, HID:], 1.0)
            ohb = ohp.tile([128, 128], BF16, tag="ohb", name="ohb")
            nc.vector.tensor_scalar(
                out=ohb[:], in0=iota[:],
                scalar1=sb_batch[:, j:j + 1], scalar2=None, op0=AL.is_equal)
            nc.tensor.matmul(out=pool_ps,
                             lhsT=ohb[:],
                             rhs=h2e[:],
                             start=(n_pool_done[0] == 0),
                             stop=(n_pool_done[0] == NWIN - 1),
                             skip_group_check=True)
            n_pool_done[0] += 1

        def emit_gather(gi):
                calls = [c for c in gcalls if c[0] == gi]
                if not calls:
                    return (None, 0)
                gq = groups[gi][0]
                mrows = qbase[gq + 1] // NPAR
                g_lo = calls[0][2]
                g_hi = calls[-1][3]
                msgs = msgsp.tile([128, MSGS_TILES * NPAR * HID], FP8,
                                  tag="mG", name="msgs")
                assert g_hi - g_lo <= MSGS_TILES, (gi, g_hi - g_lo)
                for (_, r, k_lo, k_hi) in calls:
                    mo = (k_lo - g_lo) * NPAR * HID
                    if no_gather:
                        nc.vector.memset(
                            msgs[:, mo:mo + (k_hi - k_lo) * NPAR * HID], 0.0)
                    else:
                        nc.gpsimd.dma_gather(
                            out_ap=msgs[
                                :, mo:mo + (k_hi - k_lo) * NPAR * HID]
                            .rearrange("p (t h) -> p t h", h=NPAR * HID),
                            in_ap=t2vr[r][0:mrows, 0:NPAR * HID],
                            idxs_ap=sb_idx[:, k_lo * 8:k_hi * 8],
                            num_idxs=(k_hi - k_lo) * 128,
                            num_idxs_reg=(k_hi - k_lo) * 128,
                            elem_size=NPAR * HID, elem_step=NPAR * HID,
                            single_packet=False)
                return (msgs, g_lo)

        pend = {}
        for gi in range(min(2, len(groups))):
            pend[gi] = emit_gather(gi)
        for gi, (q, jlo, jhi) in enumerate(groups):
                if gi + 2 < len(groups):
                    pend[gi + 2] = emit_gather(gi + 2)
                msgs, g_lo = pend.pop(gi)
                for j in range(jlo, jhi):
                    tl = qb_tiles.get((gi, j), [])
                    if not tl and q != 0:
                        continue
                    ch = chainb[j % 2]
                    first = True
                    if q == 0:
                        nc.tensor.matmul(
                            out=ch,
                            lhsT=dgall[:, j * 128:(j + 1) * 128],
                            rhs=t2keep[:, j * HID:(j + 1) * HID],
                            start=True, stop=(len(tl) == 0),
                            skip_group_check=True)
                        first = False
                    for i, (k, col) in enumerate(tl):
                        oh = ohp.tile([128, 128], BF16, tag="oh", name="oh")
                        build_oh(oh[:], col, k, l2_eng(k))
                        mc = (k - g_lo) * NPAR * HID
                        nc.tensor.matmul(out=ch, lhsT=oh[:],
                                         rhs=msgs[:, mc:mc + HID],
                                         start=first,
                                         stop=(i == len(tl) - 1),
                                         skip_group_check=True)
                        first = False
                    hs = h2acc[:, j * HID:(j + 1) * HID]
                    if q == 0:
                        nc.vector.tensor_copy(hs, ch)
                    else:
                        nc.vector.tensor_tensor(out=hs, in0=hs, in1=ch,
                                                op=AL.add)
                    if q == NQ - 1:
                        finish_window(j)

        # ---- pooled partial exchange (AllGather + on-device sum) ----
        pool_sb = epp.tile([128, HID + 1], F32, tag="poolsb", name="pool_sb")
        nc.vector.tensor_copy(pool_sb[:], pool_ps)
        ar_in = dram.tile([128, HID + 1], F32)
        ag_out = dram.tile([N_CORES * 128, HID + 1], F32)
        nc.sync.dma_start(ar_in[:], pool_sb[:])
        allsb = epp.tile([128, N_CORES * (HID + 1)], F32, tag="allsb",
                         name="allsb")
        if no_collectives:
            for c in range(N_CORES):
                nc.sync.dma_start(ag_out[c * 128:(c + 1) * 128, :], ar_in[:])
        else:
            nc.gpsimd.collective_compute(
                "AllGather", AL.bypass, replica_groups=rg,
                ins=[ar_in.opt()], outs=[ag_out.opt()])
        nc.sync.dma_start(
            allsb[:].rearrange("p (c h) -> p c h", c=N_CORES),
            ag_out[:].rearrange("(c p) h -> p c h", c=N_CORES))
        sums = epp.tile([128, HID + 1], F32, tag="sums", name="sums")
        nc.vector.tensor_reduce(
            out=sums[:].rearrange("p (h s) -> p h s", s=1),
            in_=allsb[:].rearrange("p (c h) -> p h c", c=N_CORES),
            op=AL.add, axis=mybir.AxisListType.X)

        cntc = epp.tile([128, 1], F32, tag="cnt", name="cntc")
        nc.vector.tensor_scalar(out=cntc[:], in0=sums[:, HID:HID + 1],
                                scalar1=1.0, scalar2=None, op0=AL.max)
        rc = epp.tile([128, 1], F32, tag="rc", name="rc")
        nc.vector.reciprocal(rc[:], cntc[:])
        pooled = epp.tile([128, HID], F32, tag="pooled", name="pooled")
        nc.vector.tensor_scalar(out=pooled[:], in0=sums[:, :HID],
                                scalar1=rc[:, :1], scalar2=None, op0=AL.mult)
        pT2 = psM2[0:HID, 192:320]
        nc.tensor.transpose(out=pT2, in_=pooled[:], identity=ident[:])
        pooledT = epp.tile([HID, 128], F32, tag="pooledT", name="pooledT")
        nc.vector.tensor_copy(pooledT[:], pT2)
        zps = psM2[0:H2, 0:128]
        nc.tensor.matmul(out=zps, lhsT=sb_lin1W[:], rhs=pooledT[:],
                         start=True, stop=True)
        zT = epp.tile([H2, 128], F32, tag="zT", name="zT")
        nc.scalar.activation(zT[:], zps, ACTF.Relu, bias=sb_lin1b[:, :1])
        ops = psM2[0:1, 320:448]
        nc.tensor.matmul(out=ops, lhsT=sb_lin2W[:], rhs=zT[:],
                         start=True, stop=True)
        outsb = epp.tile([1, 128], F32, tag="outsb", name="outsb")
        nc.vector.tensor_scalar(out=outsb[:], in0=ops,
                                scalar1=sb_lin2b[:, :1], scalar2=None,
                                op0=AL.add)
        nc.sync.dma_start(d_out.ap(), outsb[:])

    nc.compile()
    return nc


_CACHE = {}


def kernel(**inputs) -> np.ndarray:
    in_maps, meta = _prep_inputs(inputs)
    key = (meta["N"], meta["T"], meta["DSLOT"])
    if key not in _CACHE:
        _CACHE[key] = _build_nc(meta)
    nc = _CACHE[key]
    res = run_bass_kernel_spmd(nc, in_maps, core_ids=list(range(N_CORES)))
    out = np.asarray(res.results[0]["out"], np.float32).reshape(-1)
    return out[:meta["G"]].copy()


# revision 4
# speedup vs baseline: 1.1664x; 1.0415x over previous
"""BrainGCN on 8 Trainium2 NeuronCores (Bass/Tile, SPMD) — v2.

kernel(**inputs) takes FULL unsharded inputs, returns the full (G,) output.

Sharding: N nodes in 8 contiguous shards (SH=N/8); edges live on the core
owning their dst node, grouped into 128-node dst windows.  Tiles of 128
edges are classified by (window j, src%NPAR parity r, src-chunk q) where q
indexes NQ slices of every shard (for AllGather pipelining); per-(j,r) tile
counts are equalized across cores so one SPMD program serves all 8.

Unified one-hot: both layers use per-edge coefficient w'_e = w_e *
rsqrt(deg[src]).  The t2 table is h1@W2p WITHOUT the dst dinv fold, so a
single one-hot (iota==dloc)*w' drives both the L1 aggregation (rhs=onehot,
lhsT=host-pregathered x[src] rows) and the L2 aggregation (lhsT=onehot,
rhs=per-edge gathered t2 rows).  One-hots are built twice (P1 and L2) on a
DVE/Pool/ACT mix; self-loops enter via diag(dinv) matmuls.

w' comes from host-pregathered per-edge degree-slot rows (wde) reduced on
device (bf16) + Rsqrt; all FLOPs stay on device (host only permutes/pads
input data).

L2 messages: t2 stored fp8e4 [N,64] (64B rows); SWDGE bulk dma_gather with
256B elems (4 rows), NPAR=4 src-parity classes picking the row via shifted
table views; idx16 = src//4.  The AllGather runs in NQ=4 window-aligned
chunks, each fired as soon as every core has written that slice of its t2
shard, so gathers for chunk q overlap P1 compute of later chunks.

Scatter-add = one-hot matmuls into 49 concurrently-open PSUM accumulation
chains (one per dst window, skip_group_check).  Pooling = one-hot(batch)
matmul with a ones column; partial [G, H+1] pools are AllGathered (cheaper
than AllReduce) and summed on-device; the tiny MLP head is replicated.
"""
import math
from contextlib import ExitStack

import numpy as np
import ml_dtypes

import concourse.bass as bass
import concourse.bacc as bacc
import concourse.tile as tile
import concourse.mybir as mybir
from concourse import library_config
from concourse.bass_utils import run_bass_kernel_spmd

F32 = mybir.dt.float32
BF16 = mybir.dt.bfloat16
FP8 = mybir.dt.float8e4
I16 = mybir.dt.int16
AL = mybir.AluOpType
ACTF = mybir.ActivationFunctionType
BF = ml_dtypes.bfloat16
F8 = ml_dtypes.float8_e4m3

N_CORES = 8
WIN = 128
EPS = 1e-5
NQ = 4            # AllGather chunks (window-aligned slices of each shard)
NPAR = 4          # src parity classes (fp8 rows: 256B elem = 4 rows)
CALL_TILES = 32   # tiles per dma_gather call (legacy, unused)
MSGS_TILES = 64   # tiles buffered per (q, group) msgs buffer
JBLOCK = 5        # windows per wde streaming chunk


def _prep_inputs(inputs: dict):
    x = np.asarray(inputs["x"], np.float32)
    ei = np.asarray(inputs["edge_index"])
    ew = np.asarray(inputs["edge_weight"], np.float32)
    batch = np.asarray(inputs["batch"]).astype(np.int64)
    N, IN_DIM = x.shape
    HID = np.asarray(inputs["W1"]).shape[1]
    assert N % N_CORES == 0
    SH = N // N_CORES
    NWIN = math.ceil(SH / WIN)

    # window-aligned q boundaries (in windows, then node rows of each shard).
    # Front-loaded: a tile of 128 edges straddles into the next chunk, so
    # early chunks get extra windows to keep per-q gather volumes balanced.
    wq = [0, 15, 28, 41, NWIN] if NWIN == 49 else \
        [round(q * NWIN / NQ) for q in range(NQ + 1)]
    rq = [min(w * WIN, SH) for w in wq]

    # degree rows incl self-loop weight 1 (self-loops excluded from edges)
    dstA = np.concatenate([np.asarray(ei[1]), np.arange(N)]).astype(np.int64)
    wA = np.concatenate([ew, np.ones(N, np.float32)]).astype(np.float32)
    orderA = np.argsort(dstA, kind="stable")
    dsA, wsA = dstA[orderA], wA[orderA]
    countsA = np.bincount(dstA, minlength=N)
    DSLOT = int(countsA.max())
    rowptrA = np.zeros(N + 1, np.int64)
    np.cumsum(countsA, out=rowptrA[1:])
    wdeg_full = np.zeros((N, DSLOT), np.float32)
    slotA = np.arange(len(dsA)) - rowptrA[dsA]
    wdeg_full[dsA, slotA] = wsA

    src = np.asarray(ei[0]).astype(np.int64)
    dst = np.asarray(ei[1]).astype(np.int64)
    w = ew.astype(np.float32)
    order = np.argsort(dst, kind="stable")
    ds, ss, ws = dst[order], src[order], w[order]
    qs = np.searchsorted(rq, ss % SH, side="right") - 1  # src q-chunk
    # chunk-major table layout [q][core][local row][h]: each chunk's
    # AllGather output is one contiguous block (BIR requires contiguity).
    Bq = [rq[q + 1] - rq[q] for q in range(NQ)]
    qbase = np.concatenate([[0], np.cumsum([N_CORES * b for b in Bq])])
    c_src = ss // SH
    i_src = ss % SH
    rows = (qbase[qs] + c_src * np.array(Bq)[qs]
            + (i_src - np.array(rq)[qs])).astype(np.int64)
    rs = rows % NPAR
    counts = np.bincount(dst, minlength=N)
    rowptr = np.zeros(N + 1, np.int64)
    np.cumsum(counts, out=rowptr[1:])

    # per-core (window, parity) segments, sorted by q inside
    seg = {}
    cnt = np.zeros((N_CORES, NWIN, NPAR), np.int64)
    for c in range(N_CORES):
        for j in range(NWIN):
            lo = c * SH + j * WIN
            hi = min(c * SH + (j + 1) * WIN, (c + 1) * SH)
            a, b = int(rowptr[lo]), int(rowptr[hi])
            sl = slice(a, b)
            for r in range(NPAR):
                idxr = np.nonzero(rs[sl] == r)[0]
                idxr = idxr[np.argsort(qs[sl][idxr], kind="stable")]
                seg[(c, j, r)] = (lo, idxr)
                cnt[c, j, r] = len(idxr)

    ntile = np.zeros((NWIN, NPAR), np.int64)
    for j in range(NWIN):
        for r in range(NPAR):
            ntile[j, r] = -(-int(cnt[:, j, r].max()) // 128)
    T = int(ntile.sum())

    # tile q-class = max over cores of (q of last edge in tile).  Edges are
    # placed partial-tile-FIRST (pads at the very front), so the lone
    # partial tile of each cell carries the lowest-q edges and classifies
    # early instead of inflating the last chunk.
    tclass = {}
    for j in range(NWIN):
        for r in range(NPAR):
            nt = int(ntile[j, r])
            cap = nt * 128
            for t in range(nt):
                qmax = 0
                for c in range(N_CORES):
                    lo, idxr = seg[(c, j, r)]
                    n = len(idxr)
                    off = cap - n
                    e1 = (t + 1) * 128 - 1 - off
                    if e1 < 0:
                        continue
                    e1 = min(e1, n - 1)
                    a = int(rowptr[lo])
                    qmax = max(qmax, int(qs[a + idxr[e1]]))
                tclass[(j, r, t)] = qmax

    # window-major column order (for edloc/ewt/xg/wde/wprime)
    colW = {}
    c_acc = 0
    for j in range(NWIN):
        for r in range(NPAR):
            for t in range(int(ntile[j, r])):
                colW[(j, r, t)] = c_acc
                c_acc += 1
    assert c_acc == T

    # gather order: (q, window-group, r, j, t).  One dma_gather call per
    # (q, group, r); within a (q, group) the matmul sweep is window-major so
    # each PSUM bank has at most one open accumulation chain (hardware
    # corrupts interleaved chains within a bank).  Groups are greedy runs of
    # windows holding at most MSGS_TILES q-class tiles.
    qtiles_per_win = np.zeros((NQ, NWIN), np.int64)
    for (j, r, t), q in tclass.items():
        qtiles_per_win[q, j] += 1
    groups = []            # (q, jlo, jhi)
    for q in range(NQ):
        jlo = 0
        while jlo < NWIN:
            jhi, tot = jlo, 0
            while jhi < NWIN and tot + qtiles_per_win[q, jhi] <= MSGS_TILES:
                tot += qtiles_per_win[q, jhi]
                jhi += 1
            assert jhi > jlo
            groups.append((q, jlo, jhi))
            jlo = jhi
    gorder = []
    gcalls = []            # (gi, r, k_lo, k_hi)
    qb_tiles = {}          # (gi, j) -> [(k, colW)]
    for gi, (q, jlo, jhi) in enumerate(groups):
        for r in range(NPAR):
            k_lo = len(gorder)
            for j in range(jlo, jhi):
                for t in range(int(ntile[j, r])):
                    if tclass[(j, r, t)] == q:
                        gorder.append((j, r, t))
                        qb_tiles.setdefault((gi, j), []).append(
                            (len(gorder) - 1, colW[(j, r, t)]))
            if len(gorder) > k_lo:
                gcalls.append((gi, r, k_lo, len(gorder)))
    assert len(gorder) == T
    kmap = {jrt: k for k, jrt in enumerate(gorder)}

    win_tiles = [[] for _ in range(NWIN)]
    for j in range(NWIN):
        for r in range(NPAR):
            for t in range(int(ntile[j, r])):
                win_tiles[j].append((colW[(j, r, t)], kmap[(j, r, t)]))
    tile_info = [(j, colW[(j, r, t)]) for (j, r, t) in gorder]

    # --- per-core arrays ---
    edloc = np.full((N_CORES, 128, T), 999.0, np.float32)
    ewt = np.zeros((N_CORES, 128, T), np.float32)
    xg = np.zeros((N_CORES, 128, T * IN_DIM), BF)
    wde = np.zeros((N_CORES, 128, T * DSLOT), BF)
    idx16 = np.zeros((N_CORES, 16, T * 8), np.int16)

    for c in range(N_CORES):
        for j in range(NWIN):
            for r in range(NPAR):
                lo, idxr = seg[(c, j, r)]
                a = int(rowptr[lo])
                n = len(idxr)
                nt = int(ntile[j, r])
                cap = nt * 128
                off = cap - n
                s_pad = np.zeros(cap, np.int64)
                row_pad = np.zeros(cap, np.int64)
                d_pad = np.full(cap, 999.0, np.float32)
                w_pad = np.zeros(cap, np.float32)
                s_pad[off:] = ss[a + idxr]
                row_pad[off:] = rows[a + idxr]
                d_pad[off:] = (ds[a + idxr] - lo).astype(np.float32)
                w_pad[off:] = ws[a + idxr]
                padm = d_pad > 900.0
                for t in range(nt):
                    cw = colW[(j, r, t)]
                    k = kmap[(j, r, t)]
                    blk = slice(t * 128, (t + 1) * 128)
                    pm = padm[blk]
                    edloc[c, :, cw] = d_pad[blk]
                    ewt[c, :, cw] = w_pad[blk]
                    xs = x[s_pad[blk]].astype(np.float32)
                    xs[pm] = 0.0
                    xg[c, :, cw * IN_DIM:(cw + 1) * IN_DIM] = xs.astype(BF)
                    wd = wdeg_full[s_pad[blk]].copy()
                    wd[pm] = 0.0
                    wd[pm, 0] = 1.0
                    wde[c, :, cw * DSLOT:(cw + 1) * DSLOT] = wd.astype(BF)
                    iv = (row_pad[blk] // NPAR).astype(np.int16)
                    iv[pm] = 0
                    iv2 = iv.reshape(8, 16)  # [p//16, p%16]
                    idx16[c, :, k * 8:(k + 1) * 8] = iv2.T

    PADN = NWIN * WIN

    def win_major(a2d, dt=np.float32):
        S = a2d.shape[1]
        assert a2d.shape[0] == PADN
        return np.ascontiguousarray(
            a2d.reshape(NWIN, WIN, S).transpose(1, 0, 2)
            .reshape(WIN, NWIN * S)).astype(dt)

    iota = np.tile(np.arange(128, dtype=np.float32), (128, 1))
    ident = np.eye(128, dtype=np.float32)

    in_maps = []
    for c in range(N_CORES):
        wc = np.zeros((PADN, DSLOT), np.float32)
        wc[:SH] = wdeg_full[c * SH:(c + 1) * SH]
        wc[SH:, 0] = 1.0
        bv = np.full((PADN, 1), 999.0, np.float32)
        bv[:SH, 0] = batch[c * SH:(c + 1) * SH].astype(np.float32)
        xo = np.zeros((PADN, IN_DIM), np.float32)
        xo[:SH] = x[c * SH:(c + 1) * SH]
        in_maps.append({
            "edloc": edloc[c], "ewt": ewt[c].astype(BF),
            "xg": xg[c], "wde": wde[c],
            "idx16": np.tile(idx16[c], (8, 1)),
            "xnm": win_major(xo, BF),
            "wdeg": win_major(wc, BF), "batchv": win_major(bv),
            "iota": iota.astype(BF), "ident": ident,
            "W1": np.asarray(inputs["W1"], np.float32),
            "W2": np.asarray(inputs["W2"], np.float32),
            "g1": np.asarray(inputs["bn1_gamma"], np.float32).reshape(1, HID),
            "be1": np.asarray(inputs["bn1_beta"], np.float32).reshape(1, HID),
            "m1": np.asarray(inputs["bn1_mean"], np.float32).reshape(1, HID),
            "v1": np.asarray(inputs["bn1_var"], np.float32).reshape(1, HID),
            "b1": np.asarray(inputs["b1"], np.float32).reshape(1, HID),
            "g2": np.asarray(inputs["bn2_gamma"], np.float32).reshape(1, HID),
            "be2": np.asarray(inputs["bn2_beta"], np.float32).reshape(1, HID),
            "m2": np.asarray(inputs["bn2_mean"], np.float32).reshape(1, HID),
            "v2": np.asarray(inputs["bn2_var"], np.float32).reshape(1, HID),
            "b2": np.asarray(inputs["b2"], np.float32).reshape(1, HID),
            "lin1W": np.asarray(inputs["lin1_W"], np.float32),
            "lin1b": np.asarray(inputs["lin1_b"], np.float32).reshape(-1, 1),
            "lin2W": np.asarray(inputs["lin2_W"], np.float32),
            "lin2b": np.asarray(inputs["lin2_b"], np.float32).reshape(1, 1),
        })

    meta = dict(N=N, G=128, IN_DIM=IN_DIM, HID=HID, SH=SH, NWIN=NWIN,
                DSLOT=DSLOT, T=T, wq=wq, rq=rq,
                qbase=[int(x) for x in qbase], Bq=Bq,
                win_tiles=win_tiles, tile_info=tile_info,
                gcalls=gcalls, qb_tiles=qb_tiles, groups=groups)
    return in_maps, meta


def _build_nc(meta, no_collectives=False, no_gather=False):
    N, IN_DIM, HID = meta["N"], meta["IN_DIM"], meta["HID"]
    SH, NWIN, DSLOT, T = meta["SH"], meta["NWIN"], meta["DSLOT"], meta["T"]
    wq, rq, qbase = meta["wq"], meta["rq"], meta["qbase"]
    win_tiles = meta["win_tiles"]
    tile_info = meta["tile_info"]
    gcalls, qb_tiles, groups = meta["gcalls"], meta["qb_tiles"], meta["groups"]
    H2 = HID // 2

    nc = bacc.Bacc("TRN2", target_bir_lowering=False, debug=False,
                   num_devices=N_CORES)
    d_edloc = nc.dram_tensor("edloc", [128, T], F32, kind="ExternalInput")
    d_ewt = nc.dram_tensor("ewt", [128, T], BF16, kind="ExternalInput")
    d_xg = nc.dram_tensor("xg", [128, T * IN_DIM], BF16, kind="ExternalInput")
    d_wde = nc.dram_tensor("wde", [128, T * DSLOT], BF16, kind="ExternalInput")
    d_idx = nc.dram_tensor("idx16", [128, T * 8], I16, kind="ExternalInput")
    d_xnm = nc.dram_tensor("xnm", [128, NWIN * IN_DIM], BF16,
                           kind="ExternalInput")
    d_wdeg = nc.dram_tensor("wdeg", [128, NWIN * DSLOT], BF16,
                            kind="ExternalInput")
    d_batch = nc.dram_tensor("batchv", [128, NWIN], F32, kind="ExternalInput")
    d_iota = nc.dram_tensor("iota", [128, 128], BF16, kind="ExternalInput")
    d_ident = nc.dram_tensor("ident", [128, 128], F32, kind="ExternalInput")
    d_W1 = nc.dram_tensor("W1", [IN_DIM, HID], F32, kind="ExternalInput")
    d_W2 = nc.dram_tensor("W2", [HID, HID], F32, kind="ExternalInput")
    bn_names = ["g1", "be1", "m1", "v1", "b1", "g2", "be2", "m2", "v2", "b2"]
    d_bn = {k: nc.dram_tensor(k, [1, HID], F32, kind="ExternalInput")
            for k in bn_names}
    d_lin1W = nc.dram_tensor("lin1W", [HID, H2], F32, kind="ExternalInput")
    d_lin1b = nc.dram_tensor("lin1b", [H2, 1], F32, kind="ExternalInput")
    d_lin2W = nc.dram_tensor("lin2W", [H2, 1], F32, kind="ExternalInput")
    d_lin2b = nc.dram_tensor("lin2b", [1, 1], F32, kind="ExternalInput")
    d_out = nc.dram_tensor("out", [1, 128], F32, kind="ExternalOutput")

    rg = [list(range(N_CORES))]

    with tile.TileContext(nc) as tc, ExitStack() as ctx:
        constp = ctx.enter_context(tc.tile_pool(name="const", bufs=1))
        metap = ctx.enter_context(tc.tile_pool(name="meta", bufs=1))
        wdep = ctx.enter_context(tc.tile_pool(name="wdep", bufs=2))
        msgsp = ctx.enter_context(tc.tile_pool(name="msgs", bufs=3))
        wdep1 = ctx.enter_context(tc.tile_pool(name="wdep1", bufs=1))
        ohp = ctx.enter_context(tc.tile_pool(name="oh", bufs=16))
        epp = ctx.enter_context(tc.tile_pool(name="ep", bufs=4))
        vecp = ctx.enter_context(tc.tile_pool(name="vec", bufs=1))
        psA = ctx.enter_context(tc.tile_pool(name="psA", bufs=1,
                                              space="PSUM"))
        dram = ctx.enter_context(tc.tile_pool(name="dram", bufs=1,
                                              space="DRAM"))

        nc.gpsimd.load_library(library_config.mlp)

        # ---- constants and small inputs ----
        iota = constp.tile([128, 128], BF16)
        nc.sync.dma_start(iota[:], d_iota.ap())
        ident = constp.tile([128, 128], F32)
        nc.sync.dma_start(ident[:], d_ident.ap())
        ones1 = constp.tile([1, 128], F32)
        nc.vector.memset(ones1[:], 1.0)

        sb_edloc = metap.tile([128, T], F32)
        nc.sync.dma_start(sb_edloc[:], d_edloc.ap())
        sb_ewt = metap.tile([128, T], BF16)
        nc.sync.dma_start(sb_ewt[:], d_ewt.ap())
        sb_xg = metap.tile([128, T * IN_DIM], BF16)
        nc.sync.dma_start(sb_xg[:], d_xg.ap())
        sb_idx = metap.tile([128, T * 8], I16)
        nc.sync.dma_start(sb_idx[:], d_idx.ap())
        sb_xnm = metap.tile([128, NWIN * IN_DIM], BF16)
        nc.sync.dma_start(sb_xnm[:], d_xnm.ap())
        sb_wdeg = metap.tile([128, NWIN * DSLOT], BF16)
        nc.sync.dma_start(sb_wdeg[:], d_wdeg.ap())
        sb_batch = metap.tile([128, NWIN], F32)
        nc.sync.dma_start(sb_batch[:], d_batch.ap())
        sb_W1 = constp.tile([IN_DIM, HID], F32)
        nc.sync.dma_start(sb_W1[:], d_W1.ap())
        sb_W2 = constp.tile([HID, HID], F32)
        nc.sync.dma_start(sb_W2[:], d_W2.ap())
        sb_bn = {}
        for k in bn_names:
            sb_bn[k] = vecp.tile([1, HID], F32, tag=k, name="sb_" + k)
            nc.sync.dma_start(sb_bn[k][:], d_bn[k].ap())
        sb_lin1W = constp.tile([HID, H2], F32)
        sb_lin1b = constp.tile([H2, 1], F32)
        sb_lin2W = constp.tile([H2, 1], F32)
        sb_lin2b = constp.tile([1, 1], F32)
        nc.sync.dma_start(sb_lin1W[:], d_lin1W.ap())
        nc.sync.dma_start(sb_lin1b[:], d_lin1b.ap())
        nc.sync.dma_start(sb_lin2W[:], d_lin2W.ap())
        nc.sync.dma_start(sb_lin2b[:], d_lin2b.ap())

        # ---- BN folds ----
        def bn_fold(g, be, m, v, b):
            bns = vecp.tile([1, HID], F32, tag="bns" + g, name="bns" + g)
            nc.vector.tensor_scalar(out=bns[:], in0=sb_bn[v][:], scalar1=EPS,
                                    scalar2=None, op0=AL.add)
            nc.scalar.activation(bns[:], bns[:], ACTF.Sqrt)
            nc.vector.reciprocal(bns[:], bns[:])
            nc.vector.tensor_tensor(out=bns[:], in0=bns[:], in1=sb_bn[g][:],
                                    op=AL.mult)
            cc = vecp.tile([1, HID], F32, tag="c" + g, name="c" + g)
            nc.vector.tensor_tensor(out=cc[:], in0=sb_bn[b][:],
                                    in1=sb_bn[m][:], op=AL.subtract)
            nc.vector.tensor_tensor(out=cc[:], in0=cc[:], in1=bns[:],
                                    op=AL.mult)
            nc.vector.tensor_tensor(out=cc[:], in0=cc[:], in1=sb_bn[be][:],
                                    op=AL.add)
            return bns, cc

        bns1, c1v = bn_fold("g1", "be1", "m1", "v1", "b1")
        bns2, c2v = bn_fold("g2", "be2", "m2", "v2", "b2")

        # PSUM banks: two rotating L2 chain banks (one open accumulation
        # chain per bank at a time — interleaved chains within a bank are
        # corrupted by hardware), a pooling bank, and two P1 transient banks.
        chainb = [psA.tile([128, HID], F32, tag=f"ch{i}", name=f"ch{i}")[:]
                  for i in range(2)]
        poolbk = psA.tile([128, HID + 1], F32, tag="poolb", name="poolb")
        pool_ps = poolbk[:]
        psM1 = psA.tile([128, 512], F32, tag="psM1", name="psM1")
        psM2 = psA.tile([128, 512], F32, tag="psM2", name="psM2")

        def bcast128(vec, tag):
            ps = psM2[:, 128:128 + HID]
            nc.tensor.matmul(out=ps, lhsT=ones1[:], rhs=vec[:],
                             start=True, stop=True)
            sb = constp.tile([128, HID], F32, tag=tag, name="sb" + tag)
            nc.vector.tensor_copy(sb[:], ps)
            return sb

        c1_b = bcast128(c1v, "c1b")
        c2_b = bcast128(c2v, "c2b")

        def wfold(sb_W, bns, parts, tag):
            one_r = constp.tile([1, parts], F32, tag="oner" + tag,
                                name="oner" + tag)
            nc.vector.memset(one_r[:], 1.0)
            ps = psM2[0:parts, 128:128 + HID]
            nc.tensor.matmul(out=ps, lhsT=one_r[:], rhs=bns[:],
                             start=True, stop=True)
            wp = constp.tile([parts, HID], F32, tag="wp" + tag,
                             name="wp" + tag)
            nc.vector.tensor_tensor(out=wp[:], in0=sb_W[:], in1=ps[:],
                                    op=AL.mult)
            return wp

        W1p = wfold(sb_W1, bns1, IN_DIM, "1")
        W2p = wfold(sb_W2, bns2, HID, "2")

        # ---- dst dinv + persisted diag / pooling one-hots ----
        dinv = constp.tile([128, NWIN], F32)
        with nc.allow_low_precision(reason="deg sums fit bf16"):
            degd = epp.tile([128, NWIN], BF16, tag="degd", name="degd")
            nc.vector.tensor_reduce(
                out=degd[:].rearrange("p (j s) -> p j s", s=1),
                in_=sb_wdeg[:].rearrange("p (j s) -> p j s", s=DSLOT),
                op=AL.add, axis=mybir.AxisListType.X)
        nc.scalar.activation(dinv[:], degd[:], ACTF.Sqrt)
        nc.vector.reciprocal(dinv[:], dinv[:])

        dgall = constp.tile([128, NWIN * 128], BF16)

        # ---- w' = ewt * rsqrt(deg[src]) (streamed with P1 below) ----
        wprime = metap.tile([128, T], F32)
        negdloc = metap.tile([128, T], F32)
        negwp = metap.tile([128, T], F32)
        nc.vector.tensor_scalar(out=negdloc[:], in0=sb_edloc[:],
                                scalar1=-1.0, scalar2=None, op0=AL.mult)

        # t2 table (fp8) in DRAM
        t2_sh = dram.tile([SH, HID], FP8)
        t2_full = dram.tile([N + NPAR, HID], FP8)
        t2flat = t2_full[:].rearrange("n h -> (n h)")
        NROW4 = N // NPAR
        t2vr = [t2flat[r * HID: r * HID + NROW4 * NPAR * HID]
                .rearrange("(m k) -> m k", k=NPAR * HID)
                for r in range(NPAR)]
        zrow = constp.tile([NPAR, HID], FP8)
        nc.vector.memset(zrow[:], 0.0)
        nc.sync.dma_start(t2_full[N:N + NPAR, :], zrow[:])

        t2keep = metap.tile([128, NWIN * HID], BF16)
        h2acc = metap.tile([128, NWIN * HID], F32)

        # one-hot build engine schedule.  Pool builds only in early P1
        # windows (its in-order queue must reach the L2 gathers before AG0
        # completes); ACT carries a slice everywhere; DVE the rest.
        def build_oh(dst_ap, col, k, eng):
            if eng == "A":
                tt = ohp.tile([128, 128], BF16, tag="att", name="att")
                nc.scalar.activation(tt[:], iota[:], ACTF.Abs,
                                     bias=negdloc[:, col:col + 1])
                nc.scalar.activation(dst_ap, tt[:], ACTF.Relu,
                                     bias=wprime[:, col:col + 1],
                                     scale=negwp[:, col:col + 1])
            elif eng == "P":
                nc.gpsimd.tensor_scalar(
                    out=dst_ap, in0=iota[:],
                    scalar1=sb_edloc[:, col:col + 1],
                    scalar2=wprime[:, col:col + 1],
                    op0=AL.is_equal, op1=AL.mult)
            else:
                nc.vector.tensor_scalar(
                    out=dst_ap, in0=iota[:],
                    scalar1=sb_edloc[:, col:col + 1],
                    scalar2=wprime[:, col:col + 1],
                    op0=AL.is_equal, op1=AL.mult)

        def p1_eng(j, k):
            return "A" if k % 11 in (1, 6) else "D"

        def l2_eng(k):
            return "A" if k % 5 == 1 else "D"

        # ---- P1: per-block wde streaming, per-window L1 + t2, chunked AG ----
        # window-major columns: window j covers [wstart[j], wstart[j+1])
        wstart = [0] * (NWIN + 1)
        for j in range(NWIN):
            wstart[j + 1] = wstart[j] + len(win_tiles[j])
        maxbt = max(wstart[min(j + JBLOCK, NWIN)] - wstart[j]
                    for j in range(NWIN))

        q_of_block_end = {}
        for q in range(NQ):
            q_of_block_end[wq[q + 1] - 1] = q

        def p1_front(j):
            # diag one-hot JIT, then the L1 accumulation chain
            nc.vector.tensor_scalar(
                out=dgall[:, j * 128:(j + 1) * 128], in0=ident[:],
                scalar1=dinv[:, j:j + 1], scalar2=None, op0=AL.mult)
            psM = psM1 if j % 2 == 0 else psM2
            acc5 = psM[0:IN_DIM, 0:128]
            first = True
            for (col, k) in win_tiles[j]:
                oh = ohp.tile([128, 128], BF16, tag="oh", name="oh")
                build_oh(oh[:], col, k, p1_eng(j, k))
                nc.tensor.matmul(
                    out=acc5,
                    lhsT=sb_xg[:, col * IN_DIM:(col + 1) * IN_DIM],
                    rhs=oh[:], start=first, stop=False)
                first = False
            nc.tensor.matmul(
                out=acc5,
                lhsT=sb_xnm[:, j * IN_DIM:(j + 1) * IN_DIM],
                rhs=dgall[:, j * 128:(j + 1) * 128],
                start=first, stop=True)

        def p1_tail(j):
            wlen = min(WIN, SH - j * WIN)
            psM = psM1 if j % 2 == 0 else psM2
            acc5 = psM[0:IN_DIM, 0:128]
            agg5 = epp.tile([IN_DIM, 128], F32, tag="agg5", name="agg5")
            nc.scalar.activation(agg5[:], acc5, ACTF.Copy)
            ps1 = psM[:, 128:128 + HID]
            nc.tensor.matmul(out=ps1, lhsT=agg5[:], rhs=W1p[:],
                             start=True, stop=True)
            h1 = epp.tile([128, HID], F32, tag="h1", name="h1")
            nc.vector.scalar_tensor_tensor(
                out=h1[:], in0=ps1, scalar=dinv[:, j:j + 1],
                in1=c1_b[:], op0=AL.mult, op1=AL.add)
            pT = psM[0:HID, 192:320]
            nc.tensor.transpose(out=pT, in_=h1[:], identity=ident[:])
            # relu(x)^T == relu(x^T): fuse relu into the PSUM->SBUF copy
            h1T = epp.tile([HID, 128], F32, tag="h1T", name="h1T")
            nc.scalar.activation(h1T[:], pT, ACTF.Relu)
            ps2 = psM[:, 320:320 + HID]
            nc.tensor.matmul(out=ps2, lhsT=h1T[:], rhs=W2p[:],
                             start=True, stop=True)
            nc.vector.tensor_copy(t2keep[:, j * HID:(j + 1) * HID], ps2)
            nc.gpsimd.dma_start(t2_sh[j * WIN:j * WIN + wlen, :],
                                t2keep[:wlen, j * HID:(j + 1) * HID])
            if j in q_of_block_end and not no_collectives:
                q = q_of_block_end[j]
                qlo, qhi = rq[q], rq[q + 1]
                ob = qbase[q] * HID
                oe = ob + N_CORES * (qhi - qlo) * HID
                nc.gpsimd.collective_compute(
                    "AllGather", AL.bypass, replica_groups=rg,
                    ins=[t2_sh[qlo:qhi, :]],
                    outs=[t2flat[ob:oe]])

        jb = 0
        while jb < NWIN:
            je = min(jb + JBLOCK, NWIN)
            for qe in sorted(q_of_block_end):
                if jb <= qe < je:
                    je = qe + 1
                    break
            c0, c1 = wstart[jb], wstart[je]
            if c1 > c0:
                wch = wdep.tile([128, maxbt * DSLOT], BF16, tag="wde",
                                name="wch")
                cw = c1 - c0
                nc.sync.dma_start(wch[:, :cw * DSLOT],
                                  d_wde.ap()[:, c0 * DSLOT:c1 * DSLOT])
                HS1, HS2 = DSLOT // 2, DSLOT // 4
                t1 = wdep1.tile([128, maxbt * (DSLOT // 2)], BF16,
                               tag="t1", name="t1")
                with nc.allow_low_precision(reason="deg sums fit bf16"):
                    nc.vector.tensor_tensor(
                        out=t1[:, :cw * HS1].rearrange("p (j s) -> p j s",
                                                       s=HS1),
                        in0=wch[:, :cw * DSLOT].rearrange(
                            "p (j s) -> p j s", s=DSLOT)[:, :, 0:HS1],
                        in1=wch[:, :cw * DSLOT].rearrange(
                            "p (j s) -> p j s", s=DSLOT)[:, :, HS1:DSLOT],
                        op=AL.add)
                    t2t = wdep1.tile([128, maxbt * (DSLOT // 4)], BF16,
                                    tag="t2t", name="t2t")
                    nc.vector.tensor_tensor(
                        out=t2t[:, :cw * HS2].rearrange("p (j s) -> p j s",
                                                        s=HS2),
                        in0=t1[:, :cw * HS1].rearrange("p (j s) -> p j s",
                                                       s=HS1)[:, :, 0:HS2],
                        in1=t1[:, :cw * HS1].rearrange("p (j s) -> p j s",
                                                       s=HS1)[:, :, HS2:HS1],
                        op=AL.add)
                    degs = wdep1.tile([128, maxbt], BF16, tag="degs",
                                     name="degs")
                    nc.vector.tensor_reduce(
                        out=degs[:, :cw].rearrange("p (j s) -> p j s", s=1),
                        in_=t2t[:, :cw * HS2].rearrange("p (j s) -> p j s",
                                                        s=HS2),
                        op=AL.add, axis=mybir.AxisListType.X)
                rsq = wdep1.tile([128, maxbt], F32, tag="rsq",
                                name="rsq")
                nc.scalar.activation(rsq[:, :cw], degs[:, :cw], ACTF.Sqrt)
                nc.vector.reciprocal(rsq[:, :cw], rsq[:, :cw])
                nc.vector.tensor_tensor(out=wprime[:, c0:c1],
                                        in0=rsq[:, :cw],
                                        in1=sb_ewt[:, c0:c1], op=AL.mult)
                nc.vector.tensor_scalar(out=negwp[:, c0:c1],
                                        in0=wprime[:, c0:c1],
                                        scalar1=-1.0, scalar2=None,
                                        op0=AL.mult)

            for j in range(jb, je):
                p1_front(j)
                if j > 0:
                    p1_tail(j - 1)
                if j == NWIN - 1:
                    p1_tail(j)
            jb = je

        # ---- L2: per-(q, block) gathers; window-major chains + SBUF flush ----
        n_pool_done = [0]

        def finish_window(j):
            h2e = epp.tile([128, HID + 1], BF16, tag="h2e", name="h2e")
            nc.vector.scalar_tensor_tensor(
                out=h2e[:, :HID], in0=h2acc[:, j * HID:(j + 1) * HID],
                scalar=dinv[:, j:j + 1],
                in1=c2_b[:], op0=AL.mult, op1=AL.add)
            nc.scalar.activation(h2e[:, :HID], h2e[:, :HID], ACTF.Relu)
            nc.vector.memset(h2e[:, HID:], 1.0)
            ohb = ohp.tile([128, 128], BF16, tag="ohb", name="ohb")
            nc.vector.tensor_scalar(
                out=ohb[:], in0=iota[:],
                scalar1=sb_batch[:, j:j + 1], scalar2=None, op0=AL.is_equal)
            nc.tensor.matmul(out=pool_ps,
                             lhsT=ohb[:],
                             rhs=h2e[:],
                             start=(n_pool_done[0] == 0),
                             stop=(n_pool_done[0] == NWIN - 1),
                             skip_group_check=True)
            n_pool_done[0] += 1

        def emit_gather(gi):
                calls = [c for c in gcalls if c[0] == gi]
                if not calls:
                    return (None, 0)
                gq = groups[gi][0]
                mrows = qbase[gq + 1] // NPAR
                g_lo = calls[0][2]
                g_hi = calls[-1][3]
                msgs = msgsp.tile([128, MSGS_TILES * NPAR * HID], FP8,
                                  tag="mG", name="msgs")
                assert g_hi - g_lo <= MSGS_TILES, (gi, g_hi - g_lo)
                for (_, r, k_lo, k_hi) in calls:
                    mo = (k_lo - g_lo) * NPAR * HID
                    if no_gather:
                        nc.vector.memset(
                            msgs[:, mo:mo + (k_hi - k_lo) * NPAR * HID], 0.0)
                    else:
                        nc.gpsimd.dma_gather(
                            out_ap=msgs[
                                :, mo:mo + (k_hi - k_lo) * NPAR * HID]
                            .rearrange("p (t h) -> p t h", h=NPAR * HID),
                            in_ap=t2vr[r][0:mrows, 0:NPAR * HID],
                            idxs_ap=sb_idx[:, k_lo * 8:k_hi * 8],
                            num_idxs=(k_hi - k_lo) * 128,
                            num_idxs_reg=(k_hi - k_lo) * 128,
                            elem_size=NPAR * HID, elem_step=NPAR * HID,
                            single_packet=False)
                return (msgs, g_lo)

        pend = {}
        for gi in range(min(3, len(groups))):
            pend[gi] = emit_gather(gi)
        for gi, (q, jlo, jhi) in enumerate(groups):
                if gi + 3 < len(groups):
                    pend[gi + 3] = emit_gather(gi + 3)
                msgs, g_lo = pend.pop(gi)
                for j in range(jlo, jhi):
                    tl = qb_tiles.get((gi, j), [])
                    if not tl and q != 0:
                        continue
                    ch = chainb[j % 2]
                    first = True
                    if q == 0:
                        nc.tensor.matmul(
                            out=ch,
                            lhsT=dgall[:, j * 128:(j + 1) * 128],
                            rhs=t2keep[:, j * HID:(j + 1) * HID],
                            start=True, stop=(len(tl) == 0),
                            skip_group_check=True)
                        first = False
                    for i, (k, col) in enumerate(tl):
                        oh = ohp.tile([128, 128], BF16, tag="oh", name="oh")
                        build_oh(oh[:], col, k, l2_eng(k))
                        mc = (k - g_lo) * NPAR * HID
                        nc.tensor.matmul(out=ch, lhsT=oh[:],
                                         rhs=msgs[:, mc:mc + HID],
                                         start=first,
                                         stop=(i == len(tl) - 1),
                                         skip_group_check=True)
                        first = False
                    hs = h2acc[:, j * HID:(j + 1) * HID]
                    if q == 0:
                        nc.vector.tensor_copy(hs, ch)
                    else:
                        nc.vector.tensor_tensor(out=hs, in0=hs, in1=ch,
                                                op=AL.add)
                    if q == NQ - 1:
                        finish_window(j)

        # ---- pooled partial exchange (AllGather + on-device sum) ----
        pool_sb = epp.tile([128, HID + 1], F32, tag="poolsb", name="pool_sb")
        nc.vector.tensor_copy(pool_sb[:], pool_ps)
        ar_in = dram.tile([128, HID + 1], F32)
        ag_out = dram.tile([N_CORES * 128, HID + 1], F32)
        nc.sync.dma_start(ar_in[:], pool_sb[:])
        allsb = epp.tile([128, N_CORES * (HID + 1)], F32, tag="allsb",
                         name="allsb")
        if no_collectives:
            for c in range(N_CORES):
                nc.sync.dma_start(ag_out[c * 128:(c + 1) * 128, :], ar_in[:])
        else:
            nc.gpsimd.collective_compute(
                "AllGather", AL.bypass, replica_groups=rg,
                ins=[ar_in.opt()], outs=[ag_out.opt()])
        nc.sync.dma_start(
            allsb[:].rearrange("p (c h) -> p c h", c=N_CORES),
            ag_out[:].rearrange("(c p) h -> p c h", c=N_CORES))
        sums = epp.tile([128, HID + 1], F32, tag="sums", name="sums")
        nc.vector.tensor_reduce(
            out=sums[:].rearrange("p (h s) -> p h s", s=1),
            in_=allsb[:].rearrange("p (c h) -> p h c", c=N_CORES),
            op=AL.add, axis=mybir.AxisListType.X)

        cntc = epp.tile([128, 1], F32, tag="cnt", name="cntc")
        nc.vector.tensor_scalar(out=cntc[:], in0=sums[:, HID:HID + 1],
                                scalar1=1.0, scalar2=None, op0=AL.max)
        rc = epp.tile([128, 1], F32, tag="rc", name="rc")
        nc.vector.reciprocal(rc[:], cntc[:])
        pooled = epp.tile([128, HID], F32, tag="pooled", name="pooled")
        nc.vector.tensor_scalar(out=pooled[:], in0=sums[:, :HID],
                                scalar1=rc[:, :1], scalar2=None, op0=AL.mult)
        pT2 = psM2[0:HID, 192:320]
        nc.tensor.transpose(out=pT2, in_=pooled[:], identity=ident[:])
        pooledT = epp.tile([HID, 128], F32, tag="pooledT", name="pooledT")
        nc.vector.tensor_copy(pooledT[:], pT2)
        zps = psM2[0:H2, 0:128]
        nc.tensor.matmul(out=zps, lhsT=sb_lin1W[:], rhs=pooledT[:],
                         start=True, stop=True)
        zT = epp.tile([H2, 128], F32, tag="zT", name="zT")
        nc.scalar.activation(zT[:], zps, ACTF.Relu, bias=sb_lin1b[:, :1])
        ops = psM2[0:1, 320:448]
        nc.tensor.matmul(out=ops, lhsT=sb_lin2W[:], rhs=zT[:],
                         start=True, stop=True)
        outsb = epp.tile([1, 128], F32, tag="outsb", name="outsb")
        nc.vector.tensor_scalar(out=outsb[:], in0=ops,
                                scalar1=sb_lin2b[:, :1], scalar2=None,
                                op0=AL.add)
        nc.sync.dma_start(d_out.ap(), outsb[:])

    nc.compile()
    return nc


_CACHE = {}


def kernel(**inputs) -> np.ndarray:
    in_maps, meta = _prep_inputs(inputs)
    key = (meta["N"], meta["T"], meta["DSLOT"])
    if key not in _CACHE:
        _CACHE[key] = _build_nc(meta)
    nc = _CACHE[key]
    res = run_bass_kernel_spmd(nc, in_maps, core_ids=list(range(N_CORES)))
    out = np.asarray(res.results[0]["out"], np.float32).reshape(-1)
    return out[:meta["G"]].copy()


# revision 5
# speedup vs baseline: 1.1706x; 1.0036x over previous
"""BrainGCN on 8 Trainium2 NeuronCores (Bass/Tile, SPMD) — v2.

kernel(**inputs) takes FULL unsharded inputs, returns the full (G,) output.

Sharding: N nodes in 8 contiguous shards (SH=N/8); edges live on the core
owning their dst node, grouped into 128-node dst windows.  Tiles of 128
edges are classified by (window j, src%NPAR parity r, src-chunk q) where q
indexes NQ slices of every shard (for AllGather pipelining); per-(j,r) tile
counts are equalized across cores so one SPMD program serves all 8.

Unified one-hot: both layers use per-edge coefficient w'_e = w_e *
rsqrt(deg[src]).  The t2 table is h1@W2p WITHOUT the dst dinv fold, so a
single one-hot (iota==dloc)*w' drives both the L1 aggregation (rhs=onehot,
lhsT=host-pregathered x[src] rows) and the L2 aggregation (lhsT=onehot,
rhs=per-edge gathered t2 rows).  One-hots are built twice (P1 and L2) on a
DVE/Pool/ACT mix; self-loops enter via diag(dinv) matmuls.

w' comes from host-pregathered per-edge degree-slot rows (wde) reduced on
device (bf16) + Rsqrt; all FLOPs stay on device (host only permutes/pads
input data).

L2 messages: t2 stored fp8e4 [N,64] (64B rows); SWDGE bulk dma_gather with
256B elems (4 rows), NPAR=4 src-parity classes picking the row via shifted
table views; idx16 = src//4.  The AllGather runs in NQ=4 window-aligned
chunks, each fired as soon as every core has written that slice of its t2
shard, so gathers for chunk q overlap P1 compute of later chunks.

Scatter-add = one-hot matmuls into 49 concurrently-open PSUM accumulation
chains (one per dst window, skip_group_check).  Pooling = one-hot(batch)
matmul with a ones column; partial [G, H+1] pools are AllGathered (cheaper
than AllReduce) and summed on-device; the tiny MLP head is replicated.
"""
import math
from contextlib import ExitStack

import numpy as np
import ml_dtypes

import concourse.bass as bass
import concourse.bacc as bacc
import concourse.tile as tile
import concourse.mybir as mybir
from concourse import library_config
from concourse.bass_utils import run_bass_kernel_spmd

F32 = mybir.dt.float32
BF16 = mybir.dt.bfloat16
FP8 = mybir.dt.float8e4
I16 = mybir.dt.int16
AL = mybir.AluOpType
ACTF = mybir.ActivationFunctionType
BF = ml_dtypes.bfloat16
F8 = ml_dtypes.float8_e4m3

N_CORES = 8
WIN = 128
EPS = 1e-5
NQ = 4            # AllGather chunks (window-aligned slices of each shard)
NPAR = 4          # src parity classes (fp8 rows: 256B elem = 4 rows)
CALL_TILES = 32   # tiles per dma_gather call (legacy, unused)
MSGS_TILES = 64   # tiles buffered per (q, group) msgs buffer
JBLOCK = 5        # windows per wde streaming chunk


def _prep_inputs(inputs: dict):
    x = np.asarray(inputs["x"], np.float32)
    ei = np.asarray(inputs["edge_index"])
    ew = np.asarray(inputs["edge_weight"], np.float32)
    batch = np.asarray(inputs["batch"]).astype(np.int64)
    N, IN_DIM = x.shape
    HID = np.asarray(inputs["W1"]).shape[1]
    assert N % N_CORES == 0
    SH = N // N_CORES
    NWIN = math.ceil(SH / WIN)

    # window-aligned q boundaries (in windows, then node rows of each shard).
    # Front-loaded: a tile of 128 edges straddles into the next chunk, so
    # early chunks get extra windows to keep per-q gather volumes balanced.
    wq = [0, 15, 28, 41, NWIN] if NWIN == 49 else \
        [round(q * NWIN / NQ) for q in range(NQ + 1)]
    rq = [min(w * WIN, SH) for w in wq]

    # degree rows incl self-loop weight 1 (self-loops excluded from edges)
    dstA = np.concatenate([np.asarray(ei[1]), np.arange(N)]).astype(np.int64)
    wA = np.concatenate([ew, np.ones(N, np.float32)]).astype(np.float32)
    orderA = np.argsort(dstA, kind="stable")
    dsA, wsA = dstA[orderA], wA[orderA]
    countsA = np.bincount(dstA, minlength=N)
    DSLOT = int(countsA.max())
    rowptrA = np.zeros(N + 1, np.int64)
    np.cumsum(countsA, out=rowptrA[1:])
    wdeg_full = np.zeros((N, DSLOT), np.float32)
    slotA = np.arange(len(dsA)) - rowptrA[dsA]
    wdeg_full[dsA, slotA] = wsA

    src = np.asarray(ei[0]).astype(np.int64)
    dst = np.asarray(ei[1]).astype(np.int64)
    w = ew.astype(np.float32)
    order = np.argsort(dst, kind="stable")
    ds, ss, ws = dst[order], src[order], w[order]
    qs = np.searchsorted(rq, ss % SH, side="right") - 1  # src q-chunk
    # chunk-major table layout [q][core][local row][h]: each chunk's
    # AllGather output is one contiguous block (BIR requires contiguity).
    Bq = [rq[q + 1] - rq[q] for q in range(NQ)]
    qbase = np.concatenate([[0], np.cumsum([N_CORES * b for b in Bq])])
    c_src = ss // SH
    i_src = ss % SH
    rows = (qbase[qs] + c_src * np.array(Bq)[qs]
            + (i_src - np.array(rq)[qs])).astype(np.int64)
    rs = rows % NPAR
    counts = np.bincount(dst, minlength=N)
    rowptr = np.zeros(N + 1, np.int64)
    np.cumsum(counts, out=rowptr[1:])

    # per-core (window, parity) segments, sorted by q inside
    seg = {}
    cnt = np.zeros((N_CORES, NWIN, NPAR), np.int64)
    for c in range(N_CORES):
        for j in range(NWIN):
            lo = c * SH + j * WIN
            hi = min(c * SH + (j + 1) * WIN, (c + 1) * SH)
            a, b = int(rowptr[lo]), int(rowptr[hi])
            sl = slice(a, b)
            for r in range(NPAR):
                idxr = np.nonzero(rs[sl] == r)[0]
                idxr = idxr[np.argsort(qs[sl][idxr], kind="stable")]
                seg[(c, j, r)] = (lo, idxr)
                cnt[c, j, r] = len(idxr)

    ntile = np.zeros((NWIN, NPAR), np.int64)
    for j in range(NWIN):
        for r in range(NPAR):
            ntile[j, r] = -(-int(cnt[:, j, r].max()) // 128)
    T = int(ntile.sum())

    # tile q-class = max over cores of (q of last edge in tile).  Edges are
    # placed partial-tile-FIRST (pads at the very front), so the lone
    # partial tile of each cell carries the lowest-q edges and classifies
    # early instead of inflating the last chunk.
    tclass = {}
    for j in range(NWIN):
        for r in range(NPAR):
            nt = int(ntile[j, r])
            cap = nt * 128
            for t in range(nt):
                qmax = 0
                for c in range(N_CORES):
                    lo, idxr = seg[(c, j, r)]
                    n = len(idxr)
                    off = cap - n
                    e1 = (t + 1) * 128 - 1 - off
                    if e1 < 0:
                        continue
                    e1 = min(e1, n - 1)
                    a = int(rowptr[lo])
                    qmax = max(qmax, int(qs[a + idxr[e1]]))
                tclass[(j, r, t)] = qmax

    # window-major column order (for edloc/ewt/xg/wde/wprime)
    colW = {}
    c_acc = 0
    for j in range(NWIN):
        for r in range(NPAR):
            for t in range(int(ntile[j, r])):
                colW[(j, r, t)] = c_acc
                c_acc += 1
    assert c_acc == T

    # gather order: (q, window-group, r, j, t).  One dma_gather call per
    # (q, group, r); within a (q, group) the matmul sweep is window-major so
    # each PSUM bank has at most one open accumulation chain (hardware
    # corrupts interleaved chains within a bank).  Groups are greedy runs of
    # windows holding at most MSGS_TILES q-class tiles.
    qtiles_per_win = np.zeros((NQ, NWIN), np.int64)
    for (j, r, t), q in tclass.items():
        qtiles_per_win[q, j] += 1
    groups = []            # (q, jlo, jhi)
    for q in range(NQ):
        jlo = 0
        while jlo < NWIN:
            jhi, tot = jlo, 0
            while jhi < NWIN and tot + qtiles_per_win[q, jhi] <= MSGS_TILES:
                tot += qtiles_per_win[q, jhi]
                jhi += 1
            assert jhi > jlo
            groups.append((q, jlo, jhi))
            jlo = jhi
    gorder = []
    gcalls = []            # (gi, r, k_lo, k_hi)
    qb_tiles = {}          # (gi, j) -> [(k, colW)]
    for gi, (q, jlo, jhi) in enumerate(groups):
        for r in range(NPAR):
            k_lo = len(gorder)
            for j in range(jlo, jhi):
                for t in range(int(ntile[j, r])):
                    if tclass[(j, r, t)] == q:
                        gorder.append((j, r, t))
                        qb_tiles.setdefault((gi, j), []).append(
                            (len(gorder) - 1, colW[(j, r, t)]))
            if len(gorder) > k_lo:
                gcalls.append((gi, r, k_lo, len(gorder)))
    assert len(gorder) == T
    kmap = {jrt: k for k, jrt in enumerate(gorder)}

    win_tiles = [[] for _ in range(NWIN)]
    for j in range(NWIN):
        for r in range(NPAR):
            for t in range(int(ntile[j, r])):
                win_tiles[j].append((colW[(j, r, t)], kmap[(j, r, t)]))
    tile_info = [(j, colW[(j, r, t)]) for (j, r, t) in gorder]

    # --- per-core arrays ---
    edloc = np.full((N_CORES, 128, T), 999.0, np.float32)
    ewt = np.zeros((N_CORES, 128, T), np.float32)
    xg = np.zeros((N_CORES, 128, T * IN_DIM), BF)
    wde = np.zeros((N_CORES, 128, T * DSLOT), BF)
    idx16 = np.zeros((N_CORES, 16, T * 8), np.int16)

    for c in range(N_CORES):
        for j in range(NWIN):
            for r in range(NPAR):
                lo, idxr = seg[(c, j, r)]
                a = int(rowptr[lo])
                n = len(idxr)
                nt = int(ntile[j, r])
                cap = nt * 128
                off = cap - n
                s_pad = np.zeros(cap, np.int64)
                row_pad = np.zeros(cap, np.int64)
                d_pad = np.full(cap, 999.0, np.float32)
                w_pad = np.zeros(cap, np.float32)
                s_pad[off:] = ss[a + idxr]
                row_pad[off:] = rows[a + idxr]
                d_pad[off:] = (ds[a + idxr] - lo).astype(np.float32)
                w_pad[off:] = ws[a + idxr]
                padm = d_pad > 900.0
                for t in range(nt):
                    cw = colW[(j, r, t)]
                    k = kmap[(j, r, t)]
                    blk = slice(t * 128, (t + 1) * 128)
                    pm = padm[blk]
                    edloc[c, :, cw] = d_pad[blk]
                    ewt[c, :, cw] = w_pad[blk]
                    xs = x[s_pad[blk]].astype(np.float32)
                    xs[pm] = 0.0
                    xg[c, :, cw * IN_DIM:(cw + 1) * IN_DIM] = xs.astype(BF)
                    wd = wdeg_full[s_pad[blk]].copy()
                    wd[pm] = 0.0
                    wd[pm, 0] = 1.0
                    wde[c, :, cw * DSLOT:(cw + 1) * DSLOT] = wd.astype(BF)
                    iv = (row_pad[blk] // NPAR).astype(np.int16)
                    iv[pm] = 0
                    iv2 = iv.reshape(8, 16)  # [p//16, p%16]
                    idx16[c, :, k * 8:(k + 1) * 8] = iv2.T

    PADN = NWIN * WIN

    def win_major(a2d, dt=np.float32):
        S = a2d.shape[1]
        assert a2d.shape[0] == PADN
        return np.ascontiguousarray(
            a2d.reshape(NWIN, WIN, S).transpose(1, 0, 2)
            .reshape(WIN, NWIN * S)).astype(dt)

    iota = np.tile(np.arange(128, dtype=np.float32), (128, 1))
    ident = np.eye(128, dtype=np.float32)

    in_maps = []
    for c in range(N_CORES):
        wc = np.zeros((PADN, DSLOT), np.float32)
        wc[:SH] = wdeg_full[c * SH:(c + 1) * SH]
        wc[SH:, 0] = 1.0
        bv = np.full((PADN, 1), 999.0, np.float32)
        bv[:SH, 0] = batch[c * SH:(c + 1) * SH].astype(np.float32)
        xo = np.zeros((PADN, IN_DIM), np.float32)
        xo[:SH] = x[c * SH:(c + 1) * SH]
        in_maps.append({
            "edloc": edloc[c], "ewt": ewt[c].astype(BF),
            "xg": xg[c], "wde": wde[c],
            "idx16": np.tile(idx16[c], (8, 1)),
            "xnm": win_major(xo, BF),
            "wdeg": win_major(wc, BF), "batchv": win_major(bv),
            "iota": iota.astype(BF), "ident": ident,
            "W1": np.asarray(inputs["W1"], np.float32),
            "W2": np.asarray(inputs["W2"], np.float32),
            "g1": np.asarray(inputs["bn1_gamma"], np.float32).reshape(1, HID),
            "be1": np.asarray(inputs["bn1_beta"], np.float32).reshape(1, HID),
            "m1": np.asarray(inputs["bn1_mean"], np.float32).reshape(1, HID),
            "v1": np.asarray(inputs["bn1_var"], np.float32).reshape(1, HID),
            "b1": np.asarray(inputs["b1"], np.float32).reshape(1, HID),
            "g2": np.asarray(inputs["bn2_gamma"], np.float32).reshape(1, HID),
            "be2": np.asarray(inputs["bn2_beta"], np.float32).reshape(1, HID),
            "m2": np.asarray(inputs["bn2_mean"], np.float32).reshape(1, HID),
            "v2": np.asarray(inputs["bn2_var"], np.float32).reshape(1, HID),
            "b2": np.asarray(inputs["b2"], np.float32).reshape(1, HID),
            "lin1W": np.asarray(inputs["lin1_W"], np.float32),
            "lin1b": np.asarray(inputs["lin1_b"], np.float32).reshape(-1, 1),
            "lin2W": np.asarray(inputs["lin2_W"], np.float32),
            "lin2b": np.asarray(inputs["lin2_b"], np.float32).reshape(1, 1),
        })

    meta = dict(N=N, G=128, IN_DIM=IN_DIM, HID=HID, SH=SH, NWIN=NWIN,
                DSLOT=DSLOT, T=T, wq=wq, rq=rq,
                qbase=[int(x) for x in qbase], Bq=Bq,
                win_tiles=win_tiles, tile_info=tile_info,
                gcalls=gcalls, qb_tiles=qb_tiles, groups=groups)
    return in_maps, meta


def _build_nc(meta, no_collectives=False, no_gather=False):
    N, IN_DIM, HID = meta["N"], meta["IN_DIM"], meta["HID"]
    SH, NWIN, DSLOT, T = meta["SH"], meta["NWIN"], meta["DSLOT"], meta["T"]
    wq, rq, qbase = meta["wq"], meta["rq"], meta["qbase"]
    win_tiles = meta["win_tiles"]
    tile_info = meta["tile_info"]
    gcalls, qb_tiles, groups = meta["gcalls"], meta["qb_tiles"], meta["groups"]
    H2 = HID // 2

    nc = bacc.Bacc("TRN2", target_bir_lowering=False, debug=False,
                   num_devices=N_CORES)
    d_edloc = nc.dram_tensor("edloc", [128, T], F32, kind="ExternalInput")
    d_ewt = nc.dram_tensor("ewt", [128, T], BF16, kind="ExternalInput")
    d_xg = nc.dram_tensor("xg", [128, T * IN_DIM], BF16, kind="ExternalInput")
    d_wde = nc.dram_tensor("wde", [128, T * DSLOT], BF16, kind="ExternalInput")
    d_idx = nc.dram_tensor("idx16", [128, T * 8], I16, kind="ExternalInput")
    d_xnm = nc.dram_tensor("xnm", [128, NWIN * IN_DIM], BF16,
                           kind="ExternalInput")
    d_wdeg = nc.dram_tensor("wdeg", [128, NWIN * DSLOT], BF16,
                            kind="ExternalInput")
    d_batch = nc.dram_tensor("batchv", [128, NWIN], F32, kind="ExternalInput")
    d_iota = nc.dram_tensor("iota", [128, 128], BF16, kind="ExternalInput")
    d_ident = nc.dram_tensor("ident", [128, 128], F32, kind="ExternalInput")
    d_W1 = nc.dram_tensor("W1", [IN_DIM, HID], F32, kind="ExternalInput")
    d_W2 = nc.dram_tensor("W2", [HID, HID], F32, kind="ExternalInput")
    bn_names = ["g1", "be1", "m1", "v1", "b1", "g2", "be2", "m2", "v2", "b2"]
    d_bn = {k: nc.dram_tensor(k, [1, HID], F32, kind="ExternalInput")
            for k in bn_names}
    d_lin1W = nc.dram_tensor("lin1W", [HID, H2], F32, kind="ExternalInput")
    d_lin1b = nc.dram_tensor("lin1b", [H2, 1], F32, kind="ExternalInput")
    d_lin2W = nc.dram_tensor("lin2W", [H2, 1], F32, kind="ExternalInput")
    d_lin2b = nc.dram_tensor("lin2b", [1, 1], F32, kind="ExternalInput")
    d_out = nc.dram_tensor("out", [1, 128], F32, kind="ExternalOutput")

    rg = [list(range(N_CORES))]

    with tile.TileContext(nc) as tc, ExitStack() as ctx:
        constp = ctx.enter_context(tc.tile_pool(name="const", bufs=1))
        metap = ctx.enter_context(tc.tile_pool(name="meta", bufs=1))
        wdep = ctx.enter_context(tc.tile_pool(name="wdep", bufs=2))
        msgsp = ctx.enter_context(tc.tile_pool(name="msgs", bufs=3))
        wdep1 = ctx.enter_context(tc.tile_pool(name="wdep1", bufs=1))
        ohp = ctx.enter_context(tc.tile_pool(name="oh", bufs=24))
        epp = ctx.enter_context(tc.tile_pool(name="ep", bufs=4))
        vecp = ctx.enter_context(tc.tile_pool(name="vec", bufs=1))
        psA = ctx.enter_context(tc.tile_pool(name="psA", bufs=1,
                                              space="PSUM"))
        dram = ctx.enter_context(tc.tile_pool(name="dram", bufs=1,
                                              space="DRAM"))

        nc.gpsimd.load_library(library_config.mlp)

        # ---- constants and small inputs ----
        iota = constp.tile([128, 128], BF16)
        nc.sync.dma_start(iota[:], d_iota.ap())
        ident = constp.tile([128, 128], F32)
        nc.sync.dma_start(ident[:], d_ident.ap())
        ones1 = constp.tile([1, 128], F32)
        nc.vector.memset(ones1[:], 1.0)

        sb_wdeg = metap.tile([128, NWIN * DSLOT], BF16)
        nc.sync.dma_start(sb_wdeg[:], d_wdeg.ap())
        sb_edloc = metap.tile([128, T], F32)
        nc.sync.dma_start(sb_edloc[:], d_edloc.ap())
        sb_ewt = metap.tile([128, T], BF16)
        nc.sync.dma_start(sb_ewt[:], d_ewt.ap())
        sb_xnm = metap.tile([128, NWIN * IN_DIM], BF16)
        nc.sync.dma_start(sb_xnm[:], d_xnm.ap())
        sb_xg = metap.tile([128, T * IN_DIM], BF16)
        nc.sync.dma_start(sb_xg[:], d_xg.ap())
        sb_idx = metap.tile([128, T * 8], I16)
        nc.sync.dma_start(sb_idx[:], d_idx.ap())
        sb_batch = metap.tile([128, NWIN], F32)
        nc.sync.dma_start(sb_batch[:], d_batch.ap())
        sb_W1 = constp.tile([IN_DIM, HID], F32)
        nc.sync.dma_start(sb_W1[:], d_W1.ap())
        sb_W2 = constp.tile([HID, HID], F32)
        nc.sync.dma_start(sb_W2[:], d_W2.ap())
        sb_bn = {}
        for k in bn_names:
            sb_bn[k] = vecp.tile([1, HID], F32, tag=k, name="sb_" + k)
            nc.sync.dma_start(sb_bn[k][:], d_bn[k].ap())
        sb_lin1W = constp.tile([HID, H2], F32)
        sb_lin1b = constp.tile([H2, 1], F32)
        sb_lin2W = constp.tile([H2, 1], F32)
        sb_lin2b = constp.tile([1, 1], F32)
        nc.sync.dma_start(sb_lin1W[:], d_lin1W.ap())
        nc.sync.dma_start(sb_lin1b[:], d_lin1b.ap())
        nc.sync.dma_start(sb_lin2W[:], d_lin2W.ap())
        nc.sync.dma_start(sb_lin2b[:], d_lin2b.ap())

        # ---- BN folds ----
        def bn_fold(g, be, m, v, b):
            bns = vecp.tile([1, HID], F32, tag="bns" + g, name="bns" + g)
            nc.vector.tensor_scalar(out=bns[:], in0=sb_bn[v][:], scalar1=EPS,
                                    scalar2=None, op0=AL.add)
            nc.scalar.activation(bns[:], bns[:], ACTF.Sqrt)
            nc.vector.reciprocal(bns[:], bns[:])
            nc.vector.tensor_tensor(out=bns[:], in0=bns[:], in1=sb_bn[g][:],
                                    op=AL.mult)
            cc = vecp.tile([1, HID], F32, tag="c" + g, name="c" + g)
            nc.vector.tensor_tensor(out=cc[:], in0=sb_bn[b][:],
                                    in1=sb_bn[m][:], op=AL.subtract)
            nc.vector.tensor_tensor(out=cc[:], in0=cc[:], in1=bns[:],
                                    op=AL.mult)
            nc.vector.tensor_tensor(out=cc[:], in0=cc[:], in1=sb_bn[be][:],
                                    op=AL.add)
            return bns, cc

        bns1, c1v = bn_fold("g1", "be1", "m1", "v1", "b1")
        bns2, c2v = bn_fold("g2", "be2", "m2", "v2", "b2")

        # PSUM banks: two rotating L2 chain banks (one open accumulation
        # chain per bank at a time — interleaved chains within a bank are
        # corrupted by hardware), a pooling bank, and two P1 transient banks.
        chainb = [psA.tile([128, HID], F32, tag=f"ch{i}", name=f"ch{i}")[:]
                  for i in range(2)]
        poolbk = psA.tile([128, HID + 1], F32, tag="poolb", name="poolb")
        pool_ps = poolbk[:]
        psM1 = psA.tile([128, 512], F32, tag="psM1", name="psM1")
        psM2 = psA.tile([128, 512], F32, tag="psM2", name="psM2")

        def bcast128(vec, tag):
            ps = psM2[:, 128:128 + HID]
            nc.tensor.matmul(out=ps, lhsT=ones1[:], rhs=vec[:],
                             start=True, stop=True)
            sb = constp.tile([128, HID], F32, tag=tag, name="sb" + tag)
            nc.vector.tensor_copy(sb[:], ps)
            return sb

        c1_b = bcast128(c1v, "c1b")
        c2_b = bcast128(c2v, "c2b")

        def wfold(sb_W, bns, parts, tag):
            one_r = constp.tile([1, parts], F32, tag="oner" + tag,
                                name="oner" + tag)
            nc.vector.memset(one_r[:], 1.0)
            ps = psM2[0:parts, 128:128 + HID]
            nc.tensor.matmul(out=ps, lhsT=one_r[:], rhs=bns[:],
                             start=True, stop=True)
            wp = constp.tile([parts, HID], F32, tag="wp" + tag,
                             name="wp" + tag)
            nc.vector.tensor_tensor(out=wp[:], in0=sb_W[:], in1=ps[:],
                                    op=AL.mult)
            return wp

        W1p = wfold(sb_W1, bns1, IN_DIM, "1")
        W2p = wfold(sb_W2, bns2, HID, "2")

        # ---- dst dinv + persisted diag / pooling one-hots ----
        dinv = constp.tile([128, NWIN], F32)
        with nc.allow_low_precision(reason="deg sums fit bf16"):
            degd = epp.tile([128, NWIN], BF16, tag="degd", name="degd")
            nc.vector.tensor_reduce(
                out=degd[:].rearrange("p (j s) -> p j s", s=1),
                in_=sb_wdeg[:].rearrange("p (j s) -> p j s", s=DSLOT),
                op=AL.add, axis=mybir.AxisListType.X)
        nc.scalar.activation(dinv[:], degd[:], ACTF.Sqrt)
        nc.vector.reciprocal(dinv[:], dinv[:])

        dgall = constp.tile([128, NWIN * 128], BF16)

        # ---- w' = ewt * rsqrt(deg[src]) (streamed with P1 below) ----
        wprime = metap.tile([128, T], F32)
        negdloc = metap.tile([128, T], F32)
        negwp = metap.tile([128, T], F32)
        nc.vector.tensor_scalar(out=negdloc[:], in0=sb_edloc[:],
                                scalar1=-1.0, scalar2=None, op0=AL.mult)

        # t2 table (fp8) in DRAM
        t2_sh = dram.tile([SH, HID], FP8)
        t2_full = dram.tile([N + NPAR, HID], FP8)
        t2flat = t2_full[:].rearrange("n h -> (n h)")
        NROW4 = N // NPAR
        t2vr = [t2flat[r * HID: r * HID + NROW4 * NPAR * HID]
                .rearrange("(m k) -> m k", k=NPAR * HID)
                for r in range(NPAR)]
        zrow = constp.tile([NPAR, HID], FP8)
        nc.vector.memset(zrow[:], 0.0)
        nc.sync.dma_start(t2_full[N:N + NPAR, :], zrow[:])

        t2keep = metap.tile([128, NWIN * HID], BF16)
        h2acc = metap.tile([128, NWIN * HID], F32)

        # one-hot build engine schedule.  Pool builds only in early P1
        # windows (its in-order queue must reach the L2 gathers before AG0
        # completes); ACT carries a slice everywhere; DVE the rest.
        def build_oh(dst_ap, col, k, eng):
            if eng == "A":
                tt = ohp.tile([128, 128], BF16, tag="att", name="att")
                nc.scalar.activation(tt[:], iota[:], ACTF.Abs,
                                     bias=negdloc[:, col:col + 1])
                nc.scalar.activation(dst_ap, tt[:], ACTF.Relu,
                                     bias=wprime[:, col:col + 1],
                                     scale=negwp[:, col:col + 1])
            elif eng == "P":
                nc.gpsimd.tensor_scalar(
                    out=dst_ap, in0=iota[:],
                    scalar1=sb_edloc[:, col:col + 1],
                    scalar2=wprime[:, col:col + 1],
                    op0=AL.is_equal, op1=AL.mult)
            else:
                nc.vector.tensor_scalar(
                    out=dst_ap, in0=iota[:],
                    scalar1=sb_edloc[:, col:col + 1],
                    scalar2=wprime[:, col:col + 1],
                    op0=AL.is_equal, op1=AL.mult)

        def p1_eng(j, k):
            return "A" if k % 6 == 1 else "D"

        def l2_eng(k):
            return "A" if k % 5 == 1 else "D"

        # ---- P1: per-block wde streaming, per-window L1 + t2, chunked AG ----
        # window-major columns: window j covers [wstart[j], wstart[j+1])
        wstart = [0] * (NWIN + 1)
        for j in range(NWIN):
            wstart[j + 1] = wstart[j] + len(win_tiles[j])
        maxbt = max(wstart[min(j + JBLOCK, NWIN)] - wstart[j]
                    for j in range(NWIN))

        q_of_block_end = {}
        for q in range(NQ):
            q_of_block_end[wq[q + 1] - 1] = q

        def p1_front(j):
            # diag one-hot JIT, then the L1 accumulation chain
            nc.vector.tensor_scalar(
                out=dgall[:, j * 128:(j + 1) * 128], in0=ident[:],
                scalar1=dinv[:, j:j + 1], scalar2=None, op0=AL.mult)
            psM = psM1 if j % 2 == 0 else psM2
            acc5 = psM[0:IN_DIM, 0:128]
            first = True
            for (col, k) in win_tiles[j]:
                oh = ohp.tile([128, 128], BF16, tag="oh", name="oh")
                build_oh(oh[:], col, k, p1_eng(j, k))
                nc.tensor.matmul(
                    out=acc5,
                    lhsT=sb_xg[:, col * IN_DIM:(col + 1) * IN_DIM],
                    rhs=oh[:], start=first, stop=False)
                first = False
            nc.tensor.matmul(
                out=acc5,
                lhsT=sb_xnm[:, j * IN_DIM:(j + 1) * IN_DIM],
                rhs=dgall[:, j * 128:(j + 1) * 128],
                start=first, stop=True)

        def p1_tail(j):
            wlen = min(WIN, SH - j * WIN)
            psM = psM1 if j % 2 == 0 else psM2
            acc5 = psM[0:IN_DIM, 0:128]
            agg5 = epp.tile([IN_DIM, 128], F32, tag="agg5", name="agg5")
            nc.scalar.activation(agg5[:], acc5, ACTF.Copy)
            ps1 = psM[:, 128:128 + HID]
            nc.tensor.matmul(out=ps1, lhsT=agg5[:], rhs=W1p[:],
                             start=True, stop=True)
            h1 = epp.tile([128, HID], F32, tag="h1", name="h1")
            nc.vector.scalar_tensor_tensor(
                out=h1[:], in0=ps1, scalar=dinv[:, j:j + 1],
                in1=c1_b[:], op0=AL.mult, op1=AL.add)
            pT = psM[0:HID, 192:320]
            nc.tensor.transpose(out=pT, in_=h1[:], identity=ident[:])
            # relu(x)^T == relu(x^T): fuse relu into the PSUM->SBUF copy
            h1T = epp.tile([HID, 128], F32, tag="h1T", name="h1T")
            nc.scalar.activation(h1T[:], pT, ACTF.Relu)
            ps2 = psM[:, 320:320 + HID]
            nc.tensor.matmul(out=ps2, lhsT=h1T[:], rhs=W2p[:],
                             start=True, stop=True)
            nc.vector.tensor_copy(t2keep[:, j * HID:(j + 1) * HID], ps2)
            nc.gpsimd.dma_start(t2_sh[j * WIN:j * WIN + wlen, :],
                                t2keep[:wlen, j * HID:(j + 1) * HID])
            if j in q_of_block_end and not no_collectives:
                q = q_of_block_end[j]
                qlo, qhi = rq[q], rq[q + 1]
                ob = qbase[q] * HID
                oe = ob + N_CORES * (qhi - qlo) * HID
                nc.gpsimd.collective_compute(
                    "AllGather", AL.bypass, replica_groups=rg,
                    ins=[t2_sh[qlo:qhi, :]],
                    outs=[t2flat[ob:oe]])

        jb = 0
        while jb < NWIN:
            je = min(jb + JBLOCK, NWIN)
            for qe in sorted(q_of_block_end):
                if jb <= qe < je:
                    je = qe + 1
                    break
            c0, c1 = wstart[jb], wstart[je]
            if c1 > c0:
                wch = wdep.tile([128, maxbt * DSLOT], BF16, tag="wde",
                                name="wch")
                cw = c1 - c0
                nc.sync.dma_start(wch[:, :cw * DSLOT],
                                  d_wde.ap()[:, c0 * DSLOT:c1 * DSLOT])
                HS1, HS2 = DSLOT // 2, DSLOT // 4
                t1 = wdep1.tile([128, maxbt * (DSLOT // 2)], BF16,
                               tag="t1", name="t1")
                with nc.allow_low_precision(reason="deg sums fit bf16"):
                    nc.vector.tensor_tensor(
                        out=t1[:, :cw * HS1].rearrange("p (j s) -> p j s",
                                                       s=HS1),
                        in0=wch[:, :cw * DSLOT].rearrange(
                            "p (j s) -> p j s", s=DSLOT)[:, :, 0:HS1],
                        in1=wch[:, :cw * DSLOT].rearrange(
                            "p (j s) -> p j s", s=DSLOT)[:, :, HS1:DSLOT],
                        op=AL.add)
                    t2t = wdep1.tile([128, maxbt * (DSLOT // 4)], BF16,
                                    tag="t2t", name="t2t")
                    nc.vector.tensor_tensor(
                        out=t2t[:, :cw * HS2].rearrange("p (j s) -> p j s",
                                                        s=HS2),
                        in0=t1[:, :cw * HS1].rearrange("p (j s) -> p j s",
                                                       s=HS1)[:, :, 0:HS2],
                        in1=t1[:, :cw * HS1].rearrange("p (j s) -> p j s",
                                                       s=HS1)[:, :, HS2:HS1],
                        op=AL.add)
                    degs = wdep1.tile([128, maxbt], BF16, tag="degs",
                                     name="degs")
                    nc.vector.tensor_reduce(
                        out=degs[:, :cw].rearrange("p (j s) -> p j s", s=1),
                        in_=t2t[:, :cw * HS2].rearrange("p (j s) -> p j s",
                                                        s=HS2),
                        op=AL.add, axis=mybir.AxisListType.X)
                rsq = wdep1.tile([128, maxbt], F32, tag="rsq",
                                name="rsq")
                nc.scalar.activation(rsq[:, :cw], degs[:, :cw], ACTF.Sqrt)
                nc.vector.reciprocal(rsq[:, :cw], rsq[:, :cw])
                nc.vector.tensor_tensor(out=wprime[:, c0:c1],
                                        in0=rsq[:, :cw],
                                        in1=sb_ewt[:, c0:c1], op=AL.mult)
                nc.vector.tensor_scalar(out=negwp[:, c0:c1],
                                        in0=wprime[:, c0:c1],
                                        scalar1=-1.0, scalar2=None,
                                        op0=AL.mult)

            for j in range(jb, je):
                p1_front(j)
                if j > 0:
                    p1_tail(j - 1)
                if j == NWIN - 1:
                    p1_tail(j)
            jb = je

        # ---- L2: per-(q, block) gathers; window-major chains + SBUF flush ----
        n_pool_done = [0]

        def finish_window(j):
            h2e = epp.tile([128, HID + 1], BF16, tag="h2e", name="h2e")
            nc.vector.scalar_tensor_tensor(
                out=h2e[:, :HID], in0=h2acc[:, j * HID:(j + 1) * HID],
                scalar=dinv[:, j:j + 1],
                in1=c2_b[:], op0=AL.mult, op1=AL.add)
            nc.scalar.activation(h2e[:, :HID], h2e[:, :HID], ACTF.Relu)
            nc.vector.memset(h2e[:, HID:], 1.0)
            ohb = ohp.tile([128, 128], BF16, tag="ohb", name="ohb")
            nc.vector.tensor_scalar(
                out=ohb[:], in0=iota[:],
                scalar1=sb_batch[:, j:j + 1], scalar2=None, op0=AL.is_equal)
            nc.tensor.matmul(out=pool_ps,
                             lhsT=ohb[:],
                             rhs=h2e[:],
                             start=(n_pool_done[0] == 0),
                             stop=(n_pool_done[0] == NWIN - 1),
                             skip_group_check=True)
            n_pool_done[0] += 1

        def emit_gather(gi):
                calls = [c for c in gcalls if c[0] == gi]
                if not calls:
                    return (None, 0)
                gq = groups[gi][0]
                mrows = qbase[gq + 1] // NPAR
                g_lo = calls[0][2]
                g_hi = calls[-1][3]
                msgs = msgsp.tile([128, MSGS_TILES * NPAR * HID], FP8,
                                  tag="mG", name="msgs")
                assert g_hi - g_lo <= MSGS_TILES, (gi, g_hi - g_lo)
                for (_, r, k_lo, k_hi) in calls:
                    mo = (k_lo - g_lo) * NPAR * HID
                    if no_gather:
                        nc.vector.memset(
                            msgs[:, mo:mo + (k_hi - k_lo) * NPAR * HID], 0.0)
                    else:
                        nc.gpsimd.dma_gather(
                            out_ap=msgs[
                                :, mo:mo + (k_hi - k_lo) * NPAR * HID]
                            .rearrange("p (t h) -> p t h", h=NPAR * HID),
                            in_ap=t2vr[r][0:mrows, 0:NPAR * HID],
                            idxs_ap=sb_idx[:, k_lo * 8:k_hi * 8],
                            num_idxs=(k_hi - k_lo) * 128,
                            num_idxs_reg=(k_hi - k_lo) * 128,
                            elem_size=NPAR * HID, elem_step=NPAR * HID,
                            single_packet=False)
                return (msgs, g_lo)

        pend = {}
        for gi in range(min(3, len(groups))):
            pend[gi] = emit_gather(gi)
        for gi, (q, jlo, jhi) in enumerate(groups):
                if gi + 3 < len(groups):
                    pend[gi + 3] = emit_gather(gi + 3)
                msgs, g_lo = pend.pop(gi)
                for j in range(jlo, jhi):
                    tl = qb_tiles.get((gi, j), [])
                    if not tl and q != 0:
                        continue
                    ch = chainb[j % 2]
                    first = True
                    if q == 0:
                        nc.tensor.matmul(
                            out=ch,
                            lhsT=dgall[:, j * 128:(j + 1) * 128],
                            rhs=t2keep[:, j * HID:(j + 1) * HID],
                            start=True, stop=(len(tl) == 0),
                            skip_group_check=True)
                        first = False
                    for i, (k, col) in enumerate(tl):
                        oh = ohp.tile([128, 128], BF16, tag="oh", name="oh")
                        build_oh(oh[:], col, k, l2_eng(k))
                        mc = (k - g_lo) * NPAR * HID
                        nc.tensor.matmul(out=ch, lhsT=oh[:],
                                         rhs=msgs[:, mc:mc + HID],
                                         start=first,
                                         stop=(i == len(tl) - 1),
                                         skip_group_check=True)
                        first = False
                    hs = h2acc[:, j * HID:(j + 1) * HID]
                    if q == 0:
                        nc.vector.tensor_copy(hs, ch)
                    else:
                        nc.vector.tensor_tensor(out=hs, in0=hs, in1=ch,
                                                op=AL.add)
                    if q == NQ - 1:
                        finish_window(j)

        # ---- pooled partial exchange (AllGather + on-device sum) ----
        pool_sb = epp.tile([128, HID + 1], F32, tag="poolsb", name="pool_sb")
        nc.vector.tensor_copy(pool_sb[:], pool_ps)
        ar_in = dram.tile([128, HID + 1], F32)
        ag_out = dram.tile([N_CORES * 128, HID + 1], F32)
        nc.sync.dma_start(ar_in[:], pool_sb[:])
        allsb = epp.tile([128, N_CORES * (HID + 1)], F32, tag="allsb",
                         name="allsb")
        if no_collectives:
            for c in range(N_CORES):
                nc.sync.dma_start(ag_out[c * 128:(c + 1) * 128, :], ar_in[:])
        else:
            nc.gpsimd.collective_compute(
                "AllGather", AL.bypass, replica_groups=rg,
                ins=[ar_in.opt()], outs=[ag_out.opt()])
        nc.sync.dma_start(
            allsb[:].rearrange("p (c h) -> p c h", c=N_CORES),
            ag_out[:].rearrange("(c p) h -> p c h", c=N_CORES))
        sums = epp.tile([128, HID + 1], F32, tag="sums", name="sums")
        nc.vector.tensor_reduce(
            out=sums[:].rearrange("p (h s) -> p h s", s=1),
            in_=allsb[:].rearrange("p (c h) -> p h c", c=N_CORES),
            op=AL.add, axis=mybir.AxisListType.X)

        cntc = epp.tile([128, 1], F32, tag="cnt", name="cntc")
        nc.vector.tensor_scalar(out=cntc[:], in0=sums[:, HID:HID + 1],
                                scalar1=1.0, scalar2=None, op0=AL.max)
        rc = epp.tile([128, 1], F32, tag="rc", name="rc")
        nc.vector.reciprocal(rc[:], cntc[:])
        pooled = epp.tile([128, HID], F32, tag="pooled", name="pooled")
        nc.vector.tensor_scalar(out=pooled[:], in0=sums[:, :HID],
                                scalar1=rc[:, :1], scalar2=None, op0=AL.mult)
        pT2 = psM2[0:HID, 192:320]
        nc.tensor.transpose(out=pT2, in_=pooled[:], identity=ident[:])
        pooledT = epp.tile([HID, 128], F32, tag="pooledT", name="pooledT")
        nc.vector.tensor_copy(pooledT[:], pT2)
        zps = psM2[0:H2, 0:128]
        nc.tensor.matmul(out=zps, lhsT=sb_lin1W[:], rhs=pooledT[:],
                         start=True, stop=True)
        zT = epp.tile([H2, 128], F32, tag="zT", name="zT")
        nc.scalar.activation(zT[:], zps, ACTF.Relu, bias=sb_lin1b[:, :1])
        ops = psM2[0:1, 320:448]
        nc.tensor.matmul(out=ops, lhsT=sb_lin2W[:], rhs=zT[:],
                         start=True, stop=True)
        outsb = epp.tile([1, 128], F32, tag="outsb", name="outsb")
        nc.vector.tensor_scalar(out=outsb[:], in0=ops,
                                scalar1=sb_lin2b[:, :1], scalar2=None,
                                op0=AL.add)
        nc.sync.dma_start(d_out.ap(), outsb[:])

    nc.compile()
    return nc


_CACHE = {}


def kernel(**inputs) -> np.ndarray:
    in_maps, meta = _prep_inputs(inputs)
    key = (meta["N"], meta["T"], meta["DSLOT"])
    if key not in _CACHE:
        _CACHE[key] = _build_nc(meta)
    nc = _CACHE[key]
    res = run_bass_kernel_spmd(nc, in_maps, core_ids=list(range(N_CORES)))
    out = np.asarray(res.results[0]["out"], np.float32).reshape(-1)
    return out[:meta["G"]].copy()
